# revision 55
# baseline (speedup 1.0000x reference)
# Trainium2 Bass kernel: batched second-order LPC synthesis
# (frame unfold -> gain -> 11 cascaded biquads -> hann window -> overlap-add -> norm)
#
# Sharding: pure data parallel over batch. 32 batch rows / 8 cores = 4 rows per
# core; each core handles 4*1024 = 4096 frames laid out as 128 partitions x 32
# frame-blocks.
#
# Design (driven by the TimelineSim cost model + HW legality):
#  - the 11-section biquad cascade runs as a wavefront over (section, time):
#    wavefront step g updates section s at local time t = g-s+1 for all frames
#    at once with 3 elementwise ops (pair products, pair add, add-x).
#  - state is SKEWED and BLOCK-FASTEST: cell (s, t) of frame-block b sits at
#    column (t - 2s + 24)*nb + b of its lane's state tile. Block index is the
#    innermost (stride-1) AP dim of every wavefront operand, so with fp16
#    operands the DVE runs its 2x_1p packed mode (0.52 ns/elem vs 1.04).
#  - the Tile framework chains same-engine instructions through semaphores;
#    dependent back-to-back instructions pay a ~95ns (DVE) / ~62ns (Pool)
#    ack+propagate bubble. The DVE wavefront is therefore split into TWO
#    independent interleaved half-wavefronts (A/B block halves, separate
#    tiles): each instruction's producer is 2 back, hiding the latency.
#  - DVE (fp16, tensor_tensor 2x) takes DB=26 blocks as two halves; gpsimd
#    (fp32 tensor_tensor; scalar_tensor_tensor is NOT a legal Pool opcode on
#    HW, and gpsimd cannot touch PSUM) runs the remaining 6 blocks as one
#    serial wavefront (its per-instruction Q7 launch overhead outweighs the
#    smaller sem bubble, so no interleave).
#  - input staging: host-padded fp16 excitation is DMA-unfolded into a
#    double-buffered Xs tile; the gain multiply into the skewed slot-0
#    columns runs on the otherwise-idle Act engine (func=Copy,
#    scale=per-partition gain column). Chunk 0 arrives pre-unfolded and
#    gain-premultiplied from the host (t-major per lane region, DMAs issued
#    ahead of all constant loads) and is placed by one plain Act copy per
#    lane, with a tiny 8-column prefix DMA+copy per DVE lane first (the
#    wavefront needs slot-0 col g+24 at step g), so the wavefronts start
#    ~3us in while the full slabs land.
#  - output DMAs are batched: the 7 middle chunks of each batch row are
#    contiguous in the output, so one strided DMA drains them (SP.SEQ DMA
#    dispatch costs 650ns each; 9 per row serialized the tail).
#  - window + overlap-add epilogue: PE transposes each block's 128
#    final-section time-columns to PSUM (fp16 1 cyc/row); the Act engine
#    folds the hann window into its PSUM->SBUF copy (scale=win chunk) into a
#    dedicated staging buffer (DSB); the accumulate is then a packed-fp16 2x
#    tensor_tensor add into the fp16 ACC (DVE; batch row 3 on gpsimd, which
#    is idle by then). Adds rotate across batch rows so the per-row ACC RMW
#    chains interleave without sem bubbles. A no_sync scheduler barrier keeps
#    the Pool-half transposes (ready only at Pool wavefront end) BEHIND the
#    DVE-half ones in PE's in-order stream, so they can't head-of-line block
#    them (the Tile scheduler otherwise orders PE's stream by its own
#    optimistic Pool timing).
#  - output: PE transposes the fp16 accumulator back to sample-major. The
#    OLA norm of a 75%-overlap hann is exactly 2 away from the edges, and
#    that 1/2 is folded into the window scale, so the 6 interior chunks per
#    batch row are plain Act PSUM->SBUF copies; only the edge chunks need a
#    DVE multiply by 2/norm. Contiguous-row DMAs write the cropped fp32
#    result; batch rows 0-2 are emitted before the Pool-half epilogue so
#    they overlap it.
import numpy as np

HOP, WIN, PAD = 128, 512, 192
B, T, S = 32, 131072, 11
F = T // HOP          # 1024
NCORE = 8
NB = B // NCORE       # 4 batch rows per core
L = 128               # input staging chunk (columns of a frame)
NCHUNK = WIN // L     # 4
NBLK = (NB * F) // 128  # 32 frame blocks per core
NS = 11
CW = WIN + 2 * NS + 2   # 536 skewed column-groups per block
LFULL = T + 2 * PAD   # 131456
NCELL = LFULL // HOP  # 1027
ACCW = 1028
DB = 26               # frame blocks on DVE (two interleaved halves)
K0S = [1 + 128 * i for i in range(8)] + [898]  # output transpose col bases

_CACHE = {}


def _hann(n):
    return 0.5 * (1.0 - np.cos(2.0 * np.pi * np.arange(n) / n))


def _build_module(db=DB):
    import concourse.bass as bass
    import concourse.tile as tile
    from concourse import bacc, mybir
    from concourse.ap import AP

    f32 = mybir.dt.float32
    f16 = mybir.dt.float16
    mult = mybir.AluOpType.mult
    add = mybir.AluOpType.add
    copyf = mybir.ActivationFunctionType.Copy

    da = db // 2          # DVE half A blocks [0, da)
    dbb = db - da         # DVE half B blocks [da, db)
    gbn = NBLK - db       # gpsimd blocks [db, NBLK)

    nc = bacc.Bacc("TRN2", target_bir_lowering=False, debug=False)
    exh_in = nc.dram_tensor("exh", [NB, LFULL], f16, kind="ExternalInput").ap()
    # coefficients, block-fastest + section-DESCENDING:
    # col ((NS-s)*2 + pair)*nb + b, pair 0 = c2, 1 = c1
    c21a_in = nc.dram_tensor("c21a", [128, NS * da * 2], f16, kind="ExternalInput").ap()
    c21b_in = nc.dram_tensor("c21b", [128, NS * dbb * 2], f16, kind="ExternalInput").ap()
    c21p_in = nc.dram_tensor("c21p", [128, NS * gbn * 2], f32, kind="ExternalInput").ap()
    gs_in = nc.dram_tensor("gs", [128, NBLK], f32, kind="ExternalInput").ap()
    xs0_in = nc.dram_tensor("xs0", [128, L * NBLK], f16, kind="ExternalInput").ap()
    win_in = nc.dram_tensor("win4", [128, 4], f32, kind="ExternalInput").ap()
    rnt_in = nc.dram_tensor("rnt", [128, 9 * 128], f32, kind="ExternalInput").ap()
    id_in = nc.dram_tensor("idn", [128, 128], f32, kind="ExternalInput").ap()
    idh_in = nc.dram_tensor("idh", [128, 128], f16, kind="ExternalInput").ap()
    out = nc.dram_tensor("out", [NB, T], f32, kind="ExternalOutput").ap()

    with tile.TileContext(nc) as tc:
        with (
            tc.tile_pool(name="state", bufs=1) as st,
            tc.tile_pool(name="scratch", bufs=2) as sp,
            tc.tile_pool(name="psum", bufs=4, space="PSUM") as pp,
        ):
            # lanes: independent instruction streams. DVE lanes A/B are
            # interleaved per wavefront step; Pool runs one lane.
            lanes = []
            for nm, eng, nb_e, b0, dt, cin in (
                    ("a", nc.vector, da, 0, f16, c21a_in),
                    ("b", nc.vector, dbb, da, f16, c21b_in),
                    ("g", nc.gpsimd, gbn, db, f32, c21p_in)):
                H = st.tile([128, nb_e * CW], dt, tag=f"H{nm}", name=f"H{nm}")
                C21 = st.tile([128, NS * nb_e * 2], dt, tag=f"C21{nm}",
                              name=f"C21{nm}")
                lanes.append(dict(nm=nm, eng=eng, nb=nb_e, b0=b0, dt=dt,
                                  H=H, C21=C21, cin=cin))

            Xs = [st.tile([128, NBLK * L], f16, tag=f"Xs{h}", name=f"Xs{h}")
                  for h in range(2)]
            XS0M = st.tile([128, L * NBLK], f16)
            ACC = st.tile([128, NB * ACCW], f16)
            DSB = st.tile([128, 128 * 128], f16)
            GS = st.tile([128, NBLK], f32)
            WIN4 = st.tile([128, 4], f32)
            RNT = st.tile([128, 9 * 128], f32)
            IDN = st.tile([128, 128], f32)
            IDH = st.tile([128, 128], f16)

            # chunk-0 comes pre-unfolded and gain-premultiplied from the
            # host (t-major per lane region); its three per-lane DMAs go
            # first in the HWDGE queue since they gate the wavefront starts.
            xs0t = XS0M[:].tensor
            lane_off = {"a": 0, "b": L * da, "g": L * db}
            TC = 8   # tiny-prefix columns: unblocks wavefront steps 0..TC-1
            # the wavefront needs slot-0 col g+24 at step g, and the t-major
            # layout makes the first TC time-columns a contiguous prefix of
            # each lane's slab: DMA + Act-copy a tiny prefix per DVE lane
            # first so the wavefront starts while the full slabs land.
            for (off, width), e in zip(((0, L * da), (L * da, L * dbb)),
                                       (lanes[0], lanes[1])):
                nb_e = e["nb"]
                nc.sync.dma_start(
                    AP(xs0t, off, [[L * NBLK, 128], [1, TC * nb_e]]),
                    AP(xs0_in.tensor, off, [[L * NBLK, 128], [1, TC * nb_e]]))
                nc.sync.dma_start(e["C21"][:], e["cin"])
                e["eng"].memset(
                    AP(e["H"][:].tensor, 0,
                       [[e["nb"] * CW, 128], [1, 2 * NS * e["nb"]]]), 0.0)
                nc.scalar.copy(
                    AP(e["H"][:].tensor, 24 * nb_e,
                       [[nb_e * CW, 128], [1, TC * nb_e]]),
                    AP(xs0t, off, [[L * NBLK, 128], [1, TC * nb_e]]))
            # gpsimd lane: same tiny-prefix treatment (its wavefront span is
            # the critical path), then the full-slab remainder below
            e = lanes[2]
            nb_g = e["nb"]
            nc.sync.dma_start(
                AP(xs0t, L * db, [[L * NBLK, 128], [1, TC * nb_g]]),
                AP(xs0_in.tensor, L * db, [[L * NBLK, 128], [1, TC * nb_g]]))
            nc.sync.dma_start(e["C21"][:], e["cin"])
            e["eng"].memset(
                AP(e["H"][:].tensor, 0,
                   [[e["nb"] * CW, 128], [1, 2 * NS * e["nb"]]]), 0.0)
            nc.scalar.copy(
                AP(e["H"][:].tensor, 24 * nb_g,
                   [[nb_g * CW, 128], [1, TC * nb_g]]),
                AP(xs0t, L * db, [[L * NBLK, 128], [1, TC * nb_g]]))
            nc.sync.dma_start(
                AP(xs0t, L * db + TC * nb_g,
                   [[L * NBLK, 128], [1, (L - TC) * nb_g]]),
                AP(xs0_in.tensor, L * db + TC * nb_g,
                   [[L * NBLK, 128], [1, (L - TC) * nb_g]]))
            for (off, width), e in zip(((0, L * da), (L * da, L * dbb)),
                                       (lanes[0], lanes[1])):
                nb_e = e["nb"]
                nc.sync.dma_start(
                    AP(xs0t, off + TC * nb_e,
                       [[L * NBLK, 128], [1, width - TC * nb_e]]),
                    AP(xs0_in.tensor, off + TC * nb_e,
                       [[L * NBLK, 128], [1, width - TC * nb_e]]))
            nc.sync.dma_start(GS[:], gs_in)

            def lane_of(b):
                for e in lanes:
                    if e["b0"] <= b < e["b0"] + e["nb"]:
                        return e
                raise AssertionError

            # input staging: unfold DMA -> Xs, per-block Act copy with
            # per-partition gain scale into the skewed slot-0 columns
            # (cell (0, t) of block b = col (t+24)*nb + b_local).
            # staged lane-first order: gpsimd's lane is the critical path, so
            # its batch row DMAs and Act copies go first each chunk.
            def stage_dma(xst, ct, beta):
                nc.sync.dma_start(
                    AP(xst, beta * 8 * L, [[NBLK * L, 128], [1, 8 * L]]),
                    AP(exh_in.tensor,
                       beta * LFULL + ct * L,
                       [[HOP, 128], [128 * HOP, 8], [1, L]]))

            def stage_act(xst, ct, b):
                e = lane_of(b)
                nb_e = e["nb"]
                bl = b - e["b0"]
                nc.scalar.activation(
                    AP(e["H"][:].tensor, (24 + ct * L) * nb_e + bl,
                       [[nb_e * CW, 128], [nb_e, L]]),
                    AP(xst, b * L, [[NBLK * L, 128], [1, L]]),
                    copyf, scale=GS[:, b:b + 1])

            for ct in range(NCHUNK):
                xs = Xs[ct % 2]
                xst = xs[:].tensor
                if ct > 0:
                    for beta in (0, 1, 2, 3):
                        stage_dma(xst, ct, beta)
                if ct == 0:
                    # one plain Act copy per lane (gain already folded in
                    # on the host); DVE lanes first (remainder after their
                    # tiny prefix), gpsimd's full slab last
                    for e in (lanes[0], lanes[1], lanes[2]):
                        nb_e = e["nb"]
                        tc0 = TC
                        nc.scalar.copy(
                            AP(e["H"][:].tensor, (24 + tc0) * nb_e,
                               [[nb_e * CW, 128], [1, (L - tc0) * nb_e]]),
                            AP(xs0t, lane_off[e["nm"]] + tc0 * nb_e,
                               [[L * NBLK, 128], [1, (L - tc0) * nb_e]]))
                    nc.scalar.memzero(ACC[:])
                    nc.sync.dma_start(WIN4[:], win_in)
                    nc.sync.dma_start(RNT[:], rnt_in)
                    nc.sync.dma_start(IDN[:], id_in)
                    nc.sync.dma_start(IDH[:], idh_in)
                else:
                    for b in list(range(db)) + list(range(db, NBLK)):
                        stage_act(xst, ct, b)

            # wavefront ops for one lane at step g
            def lane_step(e, g):
                nm, eng, nb_e, dt = e["nm"], e["eng"], e["nb"], e["dt"]
                ht = e["H"][:].tensor
                c21t = e["C21"][:].tensor
                hc = nb_e * CW
                s_lo = max(1, g - WIN + 2)
                s_hi = min(NS, g + 1)
                ns = s_hi - s_lo + 1
                w = ns * nb_e
                off_w = g - 3 * s_hi + 25
                pr = sp.tile([128, NS * nb_e * 2], dt, tag=f"pr{nm}",
                             name=f"pr{nm}")
                t2 = sp.tile([128, NS * nb_e], dt, tag=f"t2{nm}",
                             name=f"t2{nm}")
                prt = pr[:].tensor
                t2t = t2[:].tensor
                pra = AP(prt, 0, [[NS * nb_e * 2, 128], [2 * nb_e, ns],
                                  [nb_e, 2], [1, nb_e]])
                h_pair = AP(ht, (off_w - 2) * nb_e,
                            [[hc, 128], [3 * nb_e, ns], [nb_e, 2], [1, nb_e]])
                c_pair = AP(c21t, (NS - s_hi) * 2 * nb_e,
                            [[NS * nb_e * 2, 128], [2 * nb_e, ns],
                             [nb_e, 2], [1, nb_e]])
                t2a = AP(t2t, 0, [[NS * nb_e, 128], [nb_e, ns], [1, nb_e]])
                pr_ev = AP(prt, 0, [[NS * nb_e * 2, 128], [2 * nb_e, ns],
                                    [1, nb_e]])
                pr_od = AP(prt, nb_e, [[NS * nb_e * 2, 128], [2 * nb_e, ns],
                                       [1, nb_e]])
                h_out = AP(ht, off_w * nb_e,
                           [[hc, 128], [3 * nb_e, ns], [1, nb_e]])
                h_x = AP(ht, (off_w + 2) * nb_e,
                         [[hc, 128], [3 * nb_e, ns], [1, nb_e]])
                if dt == f16:
                    yield lambda: eng.tensor_tensor(pra, h_pair, c_pair, op=mult)
                    yield lambda: eng.tensor_tensor(t2a, pr_ev, pr_od, op=add)
                    yield lambda: eng.tensor_tensor(h_out, t2a, h_x, op=add)
                else:
                    yield lambda: eng.tensor_tensor(pra, h_pair, c_pair, op=mult)
                    yield lambda: eng.tensor_tensor(t2a, pr_ev, pr_od, op=add)
                    yield lambda: eng.tensor_tensor(h_out, t2a, h_x, op=add)

            # merged wavefront loop: DVE lanes A/B interleaved per step (hides
            # the same-engine sem latency), Pool lane serial.
            for g in range(WIN + NS - 1):
                for opa, opb in zip(lane_step(lanes[0], g),
                                    lane_step(lanes[1], g)):
                    opa()
                    opb()
                for op in lane_step(lanes[2], g):
                    op()

            # overlap-add epilogue: transpose each block's 128 final (section
            # 11) time-columns to PSUM, then stt window+accumulate into ACC.
            # Batch-row rotation interleaves the per-row ACC RMW chains.
            # The DVE-half epilogue is emitted first; a no_sync scheduler
            # barrier then keeps the Pool-half transposes (which become ready
            # only at Pool wavefront end) BEHIND the DVE-half ones in PE's
            # in-order stream, so they can't head-of-line-block them.
            def rotate(blks):
                # round-robin across batch rows so consecutive adds hit
                # disjoint ACC ranges (the per-row RMW chains interleave)
                groups = {}
                for b in blks:
                    groups.setdefault(b // 8, []).append(b)
                order = []
                gs = list(groups.values())
                i = 0
                while any(gs):
                    for g in gs:
                        if i < len(g):
                            order.append(g[i])
                    i += 1
                    if all(i >= len(g) for g in gs):
                        break
                return order

            def epi_stage(j, blks, pstag, bufs):
                # PE transpose each block's window-j columns to PSUM; Act
                # folds the hann window into the PSUM->SBUF copy
                # (per-partition scale = win chunk j), pre-staged into a
                # dedicated SBUF slice so the DVE adds can run back-to-back.
                for b in blks:
                    e = lane_of(b)
                    bl = b - e["b0"]
                    nb_e = e["nb"]
                    ps = pp.tile([128, 128], e["dt"], tag=pstag, name="ps",
                                 bufs=bufs)
                    nc.tensor.transpose(
                        ps[:],
                        AP(e["H"][:].tensor, (2 + j * 128) * nb_e + bl,
                           [[nb_e * CW, 128], [nb_e, 128]]),
                        IDH[:] if e["dt"] == f16 else IDN[:])
                    sl = (j * 32 + b) * 128
                    nc.scalar.activation(DSB[:, sl:sl + 128], ps[:],
                                         copyf, scale=WIN4[:, j:j + 1])

            def epi_add(j, blks):
                for b in rotate(blks):
                    beta, bb = divmod(b, 8)
                    k0 = beta * ACCW + bb * 128 + j
                    sl = (j * 32 + b) * 128
                    nc.vector.tensor_tensor(
                        ACC[:, k0:k0 + 128], DSB[:, sl:sl + 128],
                        ACC[:, k0:k0 + 128], op=add)

            # output pass for one batch row: transpose ACC back to
            # sample-major, multiply 1/norm into a per-row staging buffer,
            # then drain with FOUR DMAs (the 7 middle chunks are contiguous
            # in the output, so one strided DMA covers them; SP.SEQ DMA
            # dispatch is 650ns each and 9 per row would serialize the tail).
            def output_beta(beta):
                otb = st.tile([128, 9 * 128], f32, tag=f"otb{beta}",
                              name=f"otb{beta}")
                o_t = otb[:].tensor
                for i, k0 in enumerate(K0S):
                    ps = pp.tile([128, 128], f16, tag="pso", name="pso",
                                 bufs=3)
                    nc.tensor.transpose(
                        ps[:], ACC[:, beta * ACCW + k0:beta * ACCW + k0 + 128],
                        IDH[:])
                    if 1 <= i <= 6:
                        nc.scalar.copy(otb[:, i * 128:(i + 1) * 128], ps[:])
                    else:
                        nc.vector.tensor_tensor(
                            otb[:, i * 128:(i + 1) * 128], ps[:],
                            RNT[:, i * 128:(i + 1) * 128], op=mult)
                    if i == 0:
                        nc.sync.dma_start(
                            AP(out.tensor, beta * T, [[1, 1], [1, 64]]),
                            AP(o_t, 64, [[9 * 128, 1], [1, 64]]))
                        nc.sync.dma_start(
                            AP(out.tensor, beta * T + 64, [[128, 127], [1, 128]]),
                            AP(o_t, 9 * 128, [[9 * 128, 127], [1, 128]]))
                    elif i == 7:
                        nc.sync.dma_start(
                            AP(out.tensor, beta * T + 129 * 128 - PAD,
                               [[128, 128], [128 * 128, 7], [1, 128]]),
                            AP(o_t, 128, [[9 * 128, 128], [128, 7], [1, 128]]))
                    elif i == 8:
                        nc.sync.dma_start(
                            AP(out.tensor, beta * T + 1025 * 128 - PAD,
                               [[1, 1], [1, 64]]),
                            AP(o_t, 8 * 128 + 127 * 9 * 128, [[9 * 128, 1], [1, 64]]))

            # DVE-half epilogue first; Pool-half (which becomes ready only
            # at Pool wavefront end) stays behind a scheduler barrier so it
            # can't head-of-line block DVE-dependent PE work.
            gblks = list(range(db, NBLK))
            for j in range(4):
                epi_stage(j, list(range(db)), "psD", 3)
                if j < 3:
                    # the Pool-half j0-j2 windows are final mid-wavefront;
                    # staging them here (PE+Act, both idle) leaves only the
                    # j3 chunk for the post-Pool tail
                    epi_stage(j, gblks, "psG", 2)
                epi_add(j, list(range(db)))
                if j < 3:
                    epi_add(j, gblks)
            # batch rows fully covered by DVE-lane blocks are complete;
            # their output overlaps the Pool-half epilogue below.
            for beta in range(db // 8):
                output_beta(beta)
            tc.no_sync_barrier()
            epi_stage(3, gblks, "psG", 2)
            epi_add(3, gblks)
            tc.no_sync_barrier()
            for beta in range(db // 8, NB):
                output_beta(beta)

    nc.compile()
    return nc


def _host_prep(ex, gain, biquads, db=DB):
    # per-core host tiles; frame n of batch row beta: p = n % 128, j = n//128,
    # block b = beta*8 + j
    f32, f16 = np.float32, np.float16
    da = db // 2
    a0 = biquads[..., 0].astype(f32)
    a1 = biquads[..., 1].astype(f32)
    a2 = biquads[..., 2].astype(f32)
    c1 = (-a1 / a0).astype(f32)          # [NB, F, S]
    c2 = (-a2 / a0).astype(f32)
    gain_eff = (gain.astype(f32) * np.prod((1.0 / a0).astype(f32), axis=-1)).astype(f32)

    c1r = c1.reshape(NB, 8, 128, S).transpose(2, 3, 0, 1).reshape(128, S, NBLK)
    c2r = c2.reshape(NB, 8, 128, S).transpose(2, 3, 0, 1).reshape(128, S, NBLK)
    C = np.stack([c2r, c1r], axis=2)[:, ::-1]     # [128, S(desc), 2, NBLK]
    C21a = np.ascontiguousarray(
        C[..., :da].reshape(128, S * 2 * da)).astype(f16)
    C21b = np.ascontiguousarray(
        C[..., da:db].reshape(128, S * 2 * (db - da))).astype(f16)
    C21p = np.ascontiguousarray(
        C[..., db:].reshape(128, S * 2 * (NBLK - db))).astype(f32)
    GSm = gain_eff.reshape(NB, 8, 128).transpose(2, 0, 1).reshape(128, NBLK)
    GSm = np.ascontiguousarray(GSm).astype(f32)
    expd = np.pad(ex.astype(f32), ((0, 0), (PAD, PAD)))
    # chunk-0 pre-unfolded + gain-premultiplied, t-major per lane region:
    # region(lane) + t*nb + bl = gain[p, b] * expd[beta, frame*HOP + t]
    arr = np.empty((128, NBLK, L), f32)
    pp_ = np.arange(128)[:, None]
    tt_ = np.arange(L)[None, :]
    for b in range(NBLK):
        arr[:, b, :] = expd[b // 8][((b % 8) * 128 + pp_) * HOP + tt_]
    arr *= GSm[:, :, None]
    xs0 = np.concatenate(
        [arr[:, lo:hi].transpose(0, 2, 1).reshape(128, -1)
         for lo, hi in ((0, da), (da, db), (db, NBLK))], axis=1)
    xs0 = np.ascontiguousarray(xs0).astype(f16)
    exh = expd.astype(f16)
    return exh, C21a, C21b, C21p, GSm, xs0


def _host_consts():
    f32 = np.float32
    win = _hann(WIN).astype(f32)
    WIN4 = np.ascontiguousarray(win.reshape(4, 128).T) * f32(0.5)
    norm = np.zeros(LFULL, f32)
    idx = (np.arange(F)[:, None] * HOP + np.arange(WIN)[None, :]).reshape(-1)
    np.add.at(norm, idx, np.broadcast_to(win, (F, WIN)).reshape(-1))
    rn = np.zeros_like(norm)
    nz = norm != 0
    rn[nz] = (f32(2.0) / norm[nz]).astype(f32)
    rn2 = rn.reshape(NCELL, 128)         # [k, p]
    RNT = np.zeros((128, 9 * 128), f32)  # [k_local, i*128 + p]
    for i, k0 in enumerate(K0S):
        RNT[:, i * 128:(i + 1) * 128] = rn2[k0:k0 + 128, :]
    IDN = np.eye(128, dtype=f32)
    return WIN4, RNT, IDN, IDN.astype(np.float16)


def _build_in_maps(ex, gain, biquads):
    WIN4, RNT, IDN, IDH = _host_consts()
    in_maps = []
    for ci in range(NCORE):
        sl = slice(ci * NB, (ci + 1) * NB)
        exh, C21a, C21b, C21p, GSm, xs0 = _host_prep(
            ex[sl], gain[sl], biquads[sl])
        in_maps.append({
            "exh": exh, "c21a": C21a, "c21b": C21b, "c21p": C21p, "gs": GSm,
            "xs0": xs0,
            "win4": WIN4, "rnt": RNT, "idn": IDN, "idh": IDH,
        })
    return in_maps


def kernel(ex, gain, biquads):
    from concourse.bass_utils import run_bass_kernel_spmd

    ex = np.asarray(ex, np.float32)
    gain = np.asarray(gain, np.float32)
    biquads = np.asarray(biquads, np.float32)

    if "nc" not in _CACHE:
        _CACHE["nc"] = _build_module()
    nc = _CACHE["nc"]

    in_maps = _build_in_maps(ex, gain, biquads)
    res = run_bass_kernel_spmd(nc, in_maps, list(range(NCORE)))
    out = np.concatenate([res.results[ci]["out"] for ci in range(NCORE)], axis=0)
    return out.astype(np.float32)


# revision 56
# speedup vs baseline: 1.0003x; 1.0003x over previous
# Trainium2 Bass kernel: batched second-order LPC synthesis
# (frame unfold -> gain -> 11 cascaded biquads -> hann window -> overlap-add -> norm)
#
# Sharding: pure data parallel over batch. 32 batch rows / 8 cores = 4 rows per
# core; each core handles 4*1024 = 4096 frames laid out as 128 partitions x 32
# frame-blocks.
#
# Design (driven by the TimelineSim cost model + HW legality):
#  - the 11-section biquad cascade runs as a wavefront over (section, time):
#    wavefront step g updates section s at local time t = g-s+1 for all frames
#    at once with 3 elementwise ops (pair products, pair add, add-x).
#  - state is SKEWED and BLOCK-FASTEST: cell (s, t) of frame-block b sits at
#    column (t - 2s + 24)*nb + b of its lane's state tile. Block index is the
#    innermost (stride-1) AP dim of every wavefront operand, so with fp16
#    operands the DVE runs its 2x_1p packed mode (0.52 ns/elem vs 1.04).
#  - the Tile framework chains same-engine instructions through semaphores;
#    dependent back-to-back instructions pay a ~95ns (DVE) / ~62ns (Pool)
#    ack+propagate bubble. The DVE wavefront is therefore split into TWO
#    independent interleaved half-wavefronts (A/B block halves, separate
#    tiles): each instruction's producer is 2 back, hiding the latency.
#  - DVE (fp16, tensor_tensor 2x) takes DB=26 blocks as two halves; gpsimd
#    (fp32 tensor_tensor; scalar_tensor_tensor is NOT a legal Pool opcode on
#    HW, and gpsimd cannot touch PSUM) runs the remaining 6 blocks as one
#    serial wavefront (its per-instruction Q7 launch overhead outweighs the
#    smaller sem bubble, so no interleave).
#  - input staging: host-padded fp16 excitation is DMA-unfolded into a
#    double-buffered Xs tile; the gain multiply into the skewed slot-0
#    columns runs on the otherwise-idle Act engine (func=Copy,
#    scale=per-partition gain column). Chunk 0 arrives pre-unfolded and
#    gain-premultiplied from the host (t-major per lane region, DMAs issued
#    ahead of all constant loads) and is placed by one plain Act copy per
#    lane, with a tiny 8-column prefix DMA+copy per DVE lane first (the
#    wavefront needs slot-0 col g+24 at step g), so the wavefronts start
#    ~3us in while the full slabs land.
#  - output DMAs are batched: the 7 middle chunks of each batch row are
#    contiguous in the output, so one strided DMA drains them (SP.SEQ DMA
#    dispatch costs 650ns each; 9 per row serialized the tail).
#  - window + overlap-add epilogue: PE transposes each block's 128
#    final-section time-columns to PSUM (fp16 1 cyc/row); the Act engine
#    folds the hann window into its PSUM->SBUF copy (scale=win chunk) into a
#    dedicated staging buffer (DSB); the accumulate is then a packed-fp16 2x
#    tensor_tensor add into the fp16 ACC (DVE; batch row 3 on gpsimd, which
#    is idle by then). Adds rotate across batch rows so the per-row ACC RMW
#    chains interleave without sem bubbles. A no_sync scheduler barrier keeps
#    the Pool-half transposes (ready only at Pool wavefront end) BEHIND the
#    DVE-half ones in PE's in-order stream, so they can't head-of-line block
#    them (the Tile scheduler otherwise orders PE's stream by its own
#    optimistic Pool timing).
#  - output: PE transposes the fp16 accumulator back to sample-major. The
#    OLA norm of a 75%-overlap hann is exactly 2 away from the edges, and
#    that 1/2 is folded into the window scale, so the 6 interior chunks per
#    batch row are plain Act PSUM->SBUF copies; only the edge chunks need a
#    DVE multiply by 2/norm. Contiguous-row DMAs write the cropped fp32
#    result; batch rows 0-2 are emitted before the Pool-half epilogue so
#    they overlap it.
import numpy as np

HOP, WIN, PAD = 128, 512, 192
B, T, S = 32, 131072, 11
F = T // HOP          # 1024
NCORE = 8
NB = B // NCORE       # 4 batch rows per core
L = 128               # input staging chunk (columns of a frame)
NCHUNK = WIN // L     # 4
NBLK = (NB * F) // 128  # 32 frame blocks per core
NS = 11
CW = WIN + 2 * NS + 2   # 536 skewed column-groups per block
LFULL = T + 2 * PAD   # 131456
NCELL = LFULL // HOP  # 1027
ACCW = 1028
DB = 26               # frame blocks on DVE (two interleaved halves)
K0S = [1 + 128 * i for i in range(8)] + [898]  # output transpose col bases

_CACHE = {}


def _hann(n):
    return 0.5 * (1.0 - np.cos(2.0 * np.pi * np.arange(n) / n))


def _build_module(db=DB):
    import concourse.bass as bass
    import concourse.tile as tile
    from concourse import bacc, mybir
    from concourse.ap import AP

    f32 = mybir.dt.float32
    f16 = mybir.dt.float16
    mult = mybir.AluOpType.mult
    add = mybir.AluOpType.add
    copyf = mybir.ActivationFunctionType.Copy

    da = db // 2          # DVE half A blocks [0, da)
    dbb = db - da         # DVE half B blocks [da, db)
    gbn = NBLK - db       # gpsimd blocks [db, NBLK)

    nc = bacc.Bacc("TRN2", target_bir_lowering=False, debug=False)
    exh_in = nc.dram_tensor("exh", [NB, LFULL], f16, kind="ExternalInput").ap()
    # coefficients, block-fastest + section-DESCENDING:
    # col ((NS-s)*2 + pair)*nb + b, pair 0 = c2, 1 = c1
    c21a_in = nc.dram_tensor("c21a", [128, NS * da * 2], f16, kind="ExternalInput").ap()
    c21b_in = nc.dram_tensor("c21b", [128, NS * dbb * 2], f16, kind="ExternalInput").ap()
    c21p_in = nc.dram_tensor("c21p", [128, NS * gbn * 2], f32, kind="ExternalInput").ap()
    gs_in = nc.dram_tensor("gs", [128, NBLK], f32, kind="ExternalInput").ap()
    xs0_in = nc.dram_tensor("xs0", [128, L * NBLK], f16, kind="ExternalInput").ap()
    win_in = nc.dram_tensor("win4", [128, 4], f32, kind="ExternalInput").ap()
    rnt_in = nc.dram_tensor("rnt", [128, 9 * 128], f32, kind="ExternalInput").ap()
    id_in = nc.dram_tensor("idn", [128, 128], f32, kind="ExternalInput").ap()
    idh_in = nc.dram_tensor("idh", [128, 128], f16, kind="ExternalInput").ap()
    out = nc.dram_tensor("out", [NB, T], f32, kind="ExternalOutput").ap()

    with tile.TileContext(nc) as tc:
        with (
            tc.tile_pool(name="state", bufs=1) as st,
            tc.tile_pool(name="scratch", bufs=2) as sp,
            tc.tile_pool(name="psum", bufs=4, space="PSUM") as pp,
        ):
            # lanes: independent instruction streams. DVE lanes A/B are
            # interleaved per wavefront step; Pool runs one lane.
            lanes = []
            for nm, eng, nb_e, b0, dt, cin in (
                    ("a", nc.vector, da, 0, f16, c21a_in),
                    ("b", nc.vector, dbb, da, f16, c21b_in),
                    ("g", nc.gpsimd, gbn, db, f32, c21p_in)):
                H = st.tile([128, nb_e * CW], dt, tag=f"H{nm}", name=f"H{nm}")
                C21 = st.tile([128, NS * nb_e * 2], dt, tag=f"C21{nm}",
                              name=f"C21{nm}")
                lanes.append(dict(nm=nm, eng=eng, nb=nb_e, b0=b0, dt=dt,
                                  H=H, C21=C21, cin=cin))

            Xs = [st.tile([128, NBLK * L], f16, tag=f"Xs{h}", name=f"Xs{h}")
                  for h in range(2)]
            XS0M = st.tile([128, L * NBLK], f16)
            ACC = st.tile([128, NB * ACCW], f16)
            DSB = st.tile([128, 128 * 128], f16)
            GS = st.tile([128, NBLK], f32)
            WIN4 = st.tile([128, 4], f32)
            RNT = st.tile([128, 9 * 128], f32)
            IDN = st.tile([128, 128], f32)
            IDH = st.tile([128, 128], f16)

            # chunk-0 comes pre-unfolded and gain-premultiplied from the
            # host (t-major per lane region); its three per-lane DMAs go
            # first in the HWDGE queue since they gate the wavefront starts.
            xs0t = XS0M[:].tensor
            lane_off = {"a": 0, "b": L * da, "g": L * db}
            TC = 8   # tiny-prefix columns: unblocks wavefront steps 0..TC-1
            # the wavefront needs slot-0 col g+24 at step g, and the t-major
            # layout makes the first TC time-columns a contiguous prefix of
            # each lane's slab: DMA + Act-copy a tiny prefix per DVE lane
            # first so the wavefront starts while the full slabs land.
            for (off, width), e in zip(((0, L * da), (L * da, L * dbb)),
                                       (lanes[0], lanes[1])):
                nb_e = e["nb"]
                nc.sync.dma_start(
                    AP(xs0t, off, [[L * NBLK, 128], [1, TC * nb_e]]),
                    AP(xs0_in.tensor, off, [[L * NBLK, 128], [1, TC * nb_e]]))
                nc.sync.dma_start(e["C21"][:], e["cin"])
                e["eng"].memset(
                    AP(e["H"][:].tensor, 0,
                       [[e["nb"] * CW, 128], [1, 2 * NS * e["nb"]]]), 0.0)
                nc.scalar.copy(
                    AP(e["H"][:].tensor, 24 * nb_e,
                       [[nb_e * CW, 128], [1, TC * nb_e]]),
                    AP(xs0t, off, [[L * NBLK, 128], [1, TC * nb_e]]))
            # gpsimd's full slab (it has schedule slack), then the DVE lanes'
            # remainders
            e = lanes[2]
            nc.sync.dma_start(
                AP(xs0t, L * db, [[L * NBLK, 128], [1, L * gbn]]),
                AP(xs0_in.tensor, L * db, [[L * NBLK, 128], [1, L * gbn]]))
            nc.sync.dma_start(e["C21"][:], e["cin"])
            e["eng"].memset(
                AP(e["H"][:].tensor, 0,
                   [[e["nb"] * CW, 128], [1, 2 * NS * e["nb"]]]), 0.0)
            for (off, width), e in zip(((0, L * da), (L * da, L * dbb)),
                                       (lanes[0], lanes[1])):
                nb_e = e["nb"]
                nc.sync.dma_start(
                    AP(xs0t, off + TC * nb_e,
                       [[L * NBLK, 128], [1, width - TC * nb_e]]),
                    AP(xs0_in.tensor, off + TC * nb_e,
                       [[L * NBLK, 128], [1, width - TC * nb_e]]))
            nc.sync.dma_start(GS[:], gs_in)

            def lane_of(b):
                for e in lanes:
                    if e["b0"] <= b < e["b0"] + e["nb"]:
                        return e
                raise AssertionError

            # input staging: unfold DMA -> Xs, per-block Act copy with
            # per-partition gain scale into the skewed slot-0 columns
            # (cell (0, t) of block b = col (t+24)*nb + b_local).
            # staged lane-first order: gpsimd's lane is the critical path, so
            # its batch row DMAs and Act copies go first each chunk.
            def stage_dma(xst, ct, beta):
                nc.sync.dma_start(
                    AP(xst, beta * 8 * L, [[NBLK * L, 128], [1, 8 * L]]),
                    AP(exh_in.tensor,
                       beta * LFULL + ct * L,
                       [[HOP, 128], [128 * HOP, 8], [1, L]]))

            def stage_act(xst, ct, b):
                e = lane_of(b)
                nb_e = e["nb"]
                bl = b - e["b0"]
                nc.scalar.activation(
                    AP(e["H"][:].tensor, (24 + ct * L) * nb_e + bl,
                       [[nb_e * CW, 128], [nb_e, L]]),
                    AP(xst, b * L, [[NBLK * L, 128], [1, L]]),
                    copyf, scale=GS[:, b:b + 1])

            for ct in range(NCHUNK):
                xs = Xs[ct % 2]
                xst = xs[:].tensor
                if ct > 0:
                    for beta in (0, 1, 2, 3):
                        stage_dma(xst, ct, beta)
                if ct == 0:
                    # one plain Act copy per lane (gain already folded in
                    # on the host); DVE lanes first (remainder after their
                    # tiny prefix), gpsimd's full slab last
                    for e in (lanes[0], lanes[1], lanes[2]):
                        nb_e = e["nb"]
                        tc0 = TC if e["nm"] in ("a", "b") else 0
                        nc.scalar.copy(
                            AP(e["H"][:].tensor, (24 + tc0) * nb_e,
                               [[nb_e * CW, 128], [1, (L - tc0) * nb_e]]),
                            AP(xs0t, lane_off[e["nm"]] + tc0 * nb_e,
                               [[L * NBLK, 128], [1, (L - tc0) * nb_e]]))
                    nc.scalar.memzero(ACC[:])
                    nc.sync.dma_start(WIN4[:], win_in)
                    nc.sync.dma_start(RNT[:], rnt_in)
                    nc.sync.dma_start(IDN[:], id_in)
                    nc.sync.dma_start(IDH[:], idh_in)
                else:
                    for b in list(range(db)) + list(range(db, NBLK)):
                        stage_act(xst, ct, b)

            # wavefront ops for one lane at step g
            def lane_step(e, g):
                nm, eng, nb_e, dt = e["nm"], e["eng"], e["nb"], e["dt"]
                ht = e["H"][:].tensor
                c21t = e["C21"][:].tensor
                hc = nb_e * CW
                s_lo = max(1, g - WIN + 2)
                s_hi = min(NS, g + 1)
                ns = s_hi - s_lo + 1
                w = ns * nb_e
                off_w = g - 3 * s_hi + 25
                pr = sp.tile([128, NS * nb_e * 2], dt, tag=f"pr{nm}",
                             name=f"pr{nm}")
                t2 = sp.tile([128, NS * nb_e], dt, tag=f"t2{nm}",
                             name=f"t2{nm}")
                prt = pr[:].tensor
                t2t = t2[:].tensor
                pra = AP(prt, 0, [[NS * nb_e * 2, 128], [2 * nb_e, ns],
                                  [nb_e, 2], [1, nb_e]])
                h_pair = AP(ht, (off_w - 2) * nb_e,
                            [[hc, 128], [3 * nb_e, ns], [nb_e, 2], [1, nb_e]])
                c_pair = AP(c21t, (NS - s_hi) * 2 * nb_e,
                            [[NS * nb_e * 2, 128], [2 * nb_e, ns],
                             [nb_e, 2], [1, nb_e]])
                t2a = AP(t2t, 0, [[NS * nb_e, 128], [nb_e, ns], [1, nb_e]])
                pr_ev = AP(prt, 0, [[NS * nb_e * 2, 128], [2 * nb_e, ns],
                                    [1, nb_e]])
                pr_od = AP(prt, nb_e, [[NS * nb_e * 2, 128], [2 * nb_e, ns],
                                       [1, nb_e]])
                h_out = AP(ht, off_w * nb_e,
                           [[hc, 128], [3 * nb_e, ns], [1, nb_e]])
                h_x = AP(ht, (off_w + 2) * nb_e,
                         [[hc, 128], [3 * nb_e, ns], [1, nb_e]])
                if dt == f16:
                    yield lambda: eng.tensor_tensor(pra, h_pair, c_pair, op=mult)
                    yield lambda: eng.tensor_tensor(t2a, pr_ev, pr_od, op=add)
                    yield lambda: eng.tensor_tensor(h_out, t2a, h_x, op=add)
                else:
                    yield lambda: eng.tensor_tensor(pra, h_pair, c_pair, op=mult)
                    yield lambda: eng.tensor_tensor(t2a, pr_ev, pr_od, op=add)
                    yield lambda: eng.tensor_tensor(h_out, t2a, h_x, op=add)

            # merged wavefront loop: DVE lanes A/B interleaved per step (hides
            # the same-engine sem latency), Pool lane serial.
            for g in range(WIN + NS - 1):
                for opa, opb in zip(lane_step(lanes[0], g),
                                    lane_step(lanes[1], g)):
                    opa()
                    opb()
                for op in lane_step(lanes[2], g):
                    op()

            # overlap-add epilogue: transpose each block's 128 final (section
            # 11) time-columns to PSUM, then stt window+accumulate into ACC.
            # Batch-row rotation interleaves the per-row ACC RMW chains.
            # The DVE-half epilogue is emitted first; a no_sync scheduler
            # barrier then keeps the Pool-half transposes (which become ready
            # only at Pool wavefront end) BEHIND the DVE-half ones in PE's
            # in-order stream, so they can't head-of-line-block them.
            def rotate(blks):
                # round-robin across batch rows so consecutive adds hit
                # disjoint ACC ranges (the per-row RMW chains interleave)
                groups = {}
                for b in blks:
                    groups.setdefault(b // 8, []).append(b)
                order = []
                gs = list(groups.values())
                i = 0
                while any(gs):
                    for g in gs:
                        if i < len(g):
                            order.append(g[i])
                    i += 1
                    if all(i >= len(g) for g in gs):
                        break
                return order

            def epi_stage(j, blks, pstag, bufs):
                # PE transpose each block's window-j columns to PSUM; Act
                # folds the hann window into the PSUM->SBUF copy
                # (per-partition scale = win chunk j), pre-staged into a
                # dedicated SBUF slice so the DVE adds can run back-to-back.
                for b in blks:
                    e = lane_of(b)
                    bl = b - e["b0"]
                    nb_e = e["nb"]
                    ps = pp.tile([128, 128], e["dt"], tag=pstag, name="ps",
                                 bufs=bufs)
                    nc.tensor.transpose(
                        ps[:],
                        AP(e["H"][:].tensor, (2 + j * 128) * nb_e + bl,
                           [[nb_e * CW, 128], [nb_e, 128]]),
                        IDH[:] if e["dt"] == f16 else IDN[:])
                    sl = (j * 32 + b) * 128
                    nc.scalar.activation(DSB[:, sl:sl + 128], ps[:],
                                         copyf, scale=WIN4[:, j:j + 1])

            def epi_add(j, blks):
                for b in rotate(blks):
                    beta, bb = divmod(b, 8)
                    k0 = beta * ACCW + bb * 128 + j
                    sl = (j * 32 + b) * 128
                    nc.vector.tensor_tensor(
                        ACC[:, k0:k0 + 128], DSB[:, sl:sl + 128],
                        ACC[:, k0:k0 + 128], op=add)

            # output pass for one batch row: transpose ACC back to
            # sample-major, multiply 1/norm into a per-row staging buffer,
            # then drain with FOUR DMAs (the 7 middle chunks are contiguous
            # in the output, so one strided DMA covers them; SP.SEQ DMA
            # dispatch is 650ns each and 9 per row would serialize the tail).
            def output_beta(beta):
                otb = st.tile([128, 9 * 128], f32, tag=f"otb{beta}",
                              name=f"otb{beta}")
                o_t = otb[:].tensor
                for i, k0 in enumerate(K0S):
                    ps = pp.tile([128, 128], f16, tag="pso", name="pso",
                                 bufs=3)
                    nc.tensor.transpose(
                        ps[:], ACC[:, beta * ACCW + k0:beta * ACCW + k0 + 128],
                        IDH[:])
                    if 1 <= i <= 6:
                        nc.scalar.copy(otb[:, i * 128:(i + 1) * 128], ps[:])
                    else:
                        nc.vector.tensor_tensor(
                            otb[:, i * 128:(i + 1) * 128], ps[:],
                            RNT[:, i * 128:(i + 1) * 128], op=mult)
                    if i == 0:
                        nc.sync.dma_start(
                            AP(out.tensor, beta * T, [[1, 1], [1, 64]]),
                            AP(o_t, 64, [[9 * 128, 1], [1, 64]]))
                        nc.sync.dma_start(
                            AP(out.tensor, beta * T + 64, [[128, 127], [1, 128]]),
                            AP(o_t, 9 * 128, [[9 * 128, 127], [1, 128]]))
                    elif i == 7:
                        nc.sync.dma_start(
                            AP(out.tensor, beta * T + 129 * 128 - PAD,
                               [[128, 128], [128 * 128, 7], [1, 128]]),
                            AP(o_t, 128, [[9 * 128, 128], [128, 7], [1, 128]]))
                    elif i == 8:
                        nc.sync.dma_start(
                            AP(out.tensor, beta * T + 1025 * 128 - PAD,
                               [[1, 1], [1, 64]]),
                            AP(o_t, 8 * 128 + 127 * 9 * 128, [[9 * 128, 1], [1, 64]]))

            # DVE-half epilogue first; Pool-half (which becomes ready only
            # at Pool wavefront end) stays behind a scheduler barrier so it
            # can't head-of-line block DVE-dependent PE work.
            gblks = list(range(db, NBLK))
            for j in range(4):
                epi_stage(j, list(range(db)), "psD", 3)
                if j < 3:
                    # the Pool-half j0-j2 windows are final mid-wavefront;
                    # staging them here (PE+Act, both idle) leaves only the
                    # j3 chunk for the post-Pool tail
                    epi_stage(j, gblks, "psG", 2)
                epi_add(j, list(range(db)))
                if j < 3:
                    epi_add(j, gblks)
            # batch rows fully covered by DVE-lane blocks are complete;
            # their output overlaps the Pool-half epilogue below.
            for beta in range(db // 8):
                output_beta(beta)
            tc.no_sync_barrier()
            epi_stage(3, gblks, "psG", 2)
            epi_add(3, gblks)
            tc.no_sync_barrier()
            for beta in range(db // 8, NB):
                output_beta(beta)

    nc.compile()
    return nc


def _host_prep(ex, gain, biquads, db=DB):
    # per-core host tiles; frame n of batch row beta: p = n % 128, j = n//128,
    # block b = beta*8 + j
    f32, f16 = np.float32, np.float16
    da = db // 2
    a0 = biquads[..., 0].astype(f32)
    a1 = biquads[..., 1].astype(f32)
    a2 = biquads[..., 2].astype(f32)
    c1 = (-a1 / a0).astype(f32)          # [NB, F, S]
    c2 = (-a2 / a0).astype(f32)
    gain_eff = (gain.astype(f32) * np.prod((1.0 / a0).astype(f32), axis=-1)).astype(f32)

    c1r = c1.reshape(NB, 8, 128, S).transpose(2, 3, 0, 1).reshape(128, S, NBLK)
    c2r = c2.reshape(NB, 8, 128, S).transpose(2, 3, 0, 1).reshape(128, S, NBLK)
    C = np.stack([c2r, c1r], axis=2)[:, ::-1]     # [128, S(desc), 2, NBLK]
    C21a = np.ascontiguousarray(
        C[..., :da].reshape(128, S * 2 * da)).astype(f16)
    C21b = np.ascontiguousarray(
        C[..., da:db].reshape(128, S * 2 * (db - da))).astype(f16)
    C21p = np.ascontiguousarray(
        C[..., db:].reshape(128, S * 2 * (NBLK - db))).astype(f32)
    GSm = gain_eff.reshape(NB, 8, 128).transpose(2, 0, 1).reshape(128, NBLK)
    GSm = np.ascontiguousarray(GSm).astype(f32)
    expd = np.pad(ex.astype(f32), ((0, 0), (PAD, PAD)))
    # chunk-0 pre-unfolded + gain-premultiplied, t-major per lane region:
    # region(lane) + t*nb + bl = gain[p, b] * expd[beta, frame*HOP + t]
    arr = np.empty((128, NBLK, L), f32)
    pp_ = np.arange(128)[:, None]
    tt_ = np.arange(L)[None, :]
    for b in range(NBLK):
        arr[:, b, :] = expd[b // 8][((b % 8) * 128 + pp_) * HOP + tt_]
    arr *= GSm[:, :, None]
    xs0 = np.concatenate(
        [arr[:, lo:hi].transpose(0, 2, 1).reshape(128, -1)
         for lo, hi in ((0, da), (da, db), (db, NBLK))], axis=1)
    xs0 = np.ascontiguousarray(xs0).astype(f16)
    exh = expd.astype(f16)
    return exh, C21a, C21b, C21p, GSm, xs0


def _host_consts():
    f32 = np.float32
    win = _hann(WIN).astype(f32)
    WIN4 = np.ascontiguousarray(win.reshape(4, 128).T) * f32(0.5)
    norm = np.zeros(LFULL, f32)
    idx = (np.arange(F)[:, None] * HOP + np.arange(WIN)[None, :]).reshape(-1)
    np.add.at(norm, idx, np.broadcast_to(win, (F, WIN)).reshape(-1))
    rn = np.zeros_like(norm)
    nz = norm != 0
    rn[nz] = (f32(2.0) / norm[nz]).astype(f32)
    rn2 = rn.reshape(NCELL, 128)         # [k, p]
    RNT = np.zeros((128, 9 * 128), f32)  # [k_local, i*128 + p]
    for i, k0 in enumerate(K0S):
        RNT[:, i * 128:(i + 1) * 128] = rn2[k0:k0 + 128, :]
    IDN = np.eye(128, dtype=f32)
    return WIN4, RNT, IDN, IDN.astype(np.float16)


def _build_in_maps(ex, gain, biquads):
    WIN4, RNT, IDN, IDH = _host_consts()
    in_maps = []
    for ci in range(NCORE):
        sl = slice(ci * NB, (ci + 1) * NB)
        exh, C21a, C21b, C21p, GSm, xs0 = _host_prep(
            ex[sl], gain[sl], biquads[sl])
        in_maps.append({
            "exh": exh, "c21a": C21a, "c21b": C21b, "c21p": C21p, "gs": GSm,
            "xs0": xs0,
            "win4": WIN4, "rnt": RNT, "idn": IDN, "idh": IDH,
        })
    return in_maps


def kernel(ex, gain, biquads):
    from concourse.bass_utils import run_bass_kernel_spmd

    ex = np.asarray(ex, np.float32)
    gain = np.asarray(gain, np.float32)
    biquads = np.asarray(biquads, np.float32)

    if "nc" not in _CACHE:
        _CACHE["nc"] = _build_module()
    nc = _CACHE["nc"]

    in_maps = _build_in_maps(ex, gain, biquads)
    res = run_bass_kernel_spmd(nc, in_maps, list(range(NCORE)))
    out = np.concatenate([res.results[ci]["out"] for ci in range(NCORE)], axis=0)
    return out.astype(np.float32)


# revision 57
# speedup vs baseline: 1.0003x; 1.0001x over previous
# Trainium2 Bass kernel: batched second-order LPC synthesis
# (frame unfold -> gain -> 11 cascaded biquads -> hann window -> overlap-add -> norm)
#
# Sharding: pure data parallel over batch. 32 batch rows / 8 cores = 4 rows per
# core; each core handles 4*1024 = 4096 frames laid out as 128 partitions x 32
# frame-blocks.
#
# Design (driven by the TimelineSim cost model + HW legality):
#  - the 11-section biquad cascade runs as a wavefront over (section, time):
#    wavefront step g updates section s at local time t = g-s+1 for all frames
#    at once with 3 elementwise ops (pair products, pair add, add-x).
#  - state is SKEWED and BLOCK-FASTEST: cell (s, t) of frame-block b sits at
#    column (t - 2s + 24)*nb + b of its lane's state tile. Block index is the
#    innermost (stride-1) AP dim of every wavefront operand, so with fp16
#    operands the DVE runs its 2x_1p packed mode (0.52 ns/elem vs 1.04).
#  - the Tile framework chains same-engine instructions through semaphores;
#    dependent back-to-back instructions pay a ~95ns (DVE) / ~62ns (Pool)
#    ack+propagate bubble. The DVE wavefront is therefore split into TWO
#    independent interleaved half-wavefronts (A/B block halves, separate
#    tiles): each instruction's producer is 2 back, hiding the latency.
#  - DVE (fp16, tensor_tensor 2x) takes DB=26 blocks as two halves; gpsimd
#    (fp32 tensor_tensor; scalar_tensor_tensor is NOT a legal Pool opcode on
#    HW, and gpsimd cannot touch PSUM) runs the remaining 6 blocks as one
#    serial wavefront (its per-instruction Q7 launch overhead outweighs the
#    smaller sem bubble, so no interleave).
#  - input staging: host-padded fp16 excitation is DMA-unfolded into a
#    double-buffered Xs tile; the gain multiply into the skewed slot-0
#    columns runs on the otherwise-idle Act engine (func=Copy,
#    scale=per-partition gain column). Chunk 0 arrives pre-unfolded and
#    gain-premultiplied from the host (t-major per lane region, DMAs issued
#    ahead of all constant loads) and is placed by one plain Act copy per
#    lane, with a tiny 8-column prefix DMA+copy per DVE lane first (the
#    wavefront needs slot-0 col g+24 at step g), so the wavefronts start
#    ~3us in while the full slabs land.
#  - output DMAs are batched: the 7 middle chunks of each batch row are
#    contiguous in the output, so one strided DMA drains them (SP.SEQ DMA
#    dispatch costs 650ns each; 9 per row serialized the tail).
#  - window + overlap-add epilogue: PE transposes each block's 128
#    final-section time-columns to PSUM (fp16 1 cyc/row); the Act engine
#    folds the hann window into its PSUM->SBUF copy (scale=win chunk) into a
#    dedicated staging buffer (DSB); the accumulate is then a packed-fp16 2x
#    tensor_tensor add into the fp16 ACC (DVE; batch row 3 on gpsimd, which
#    is idle by then). Adds rotate across batch rows so the per-row ACC RMW
#    chains interleave without sem bubbles. A no_sync scheduler barrier keeps
#    the Pool-half transposes (ready only at Pool wavefront end) BEHIND the
#    DVE-half ones in PE's in-order stream, so they can't head-of-line block
#    them (the Tile scheduler otherwise orders PE's stream by its own
#    optimistic Pool timing).
#  - output: PE transposes the fp16 accumulator back to sample-major. The
#    OLA norm of a 75%-overlap hann is exactly 2 away from the edges, and
#    that 1/2 is folded into the window scale, so the 6 interior chunks per
#    batch row are plain Act PSUM->SBUF copies; only the edge chunks need a
#    DVE multiply by 2/norm. Contiguous-row DMAs write the cropped fp32
#    result; batch rows 0-2 are emitted before the Pool-half epilogue so
#    they overlap it.
import numpy as np

HOP, WIN, PAD = 128, 512, 192
B, T, S = 32, 131072, 11
F = T // HOP          # 1024
NCORE = 8
NB = B // NCORE       # 4 batch rows per core
L = 128               # input staging chunk (columns of a frame)
NCHUNK = WIN // L     # 4
NBLK = (NB * F) // 128  # 32 frame blocks per core
NS = 11
CW = WIN + 2 * NS + 2   # 536 skewed column-groups per block
LFULL = T + 2 * PAD   # 131456
NCELL = LFULL // HOP  # 1027
ACCW = 1028
DB = 26               # frame blocks on DVE (two interleaved halves)
K0S = [1 + 128 * i for i in range(8)] + [898]  # output transpose col bases

_CACHE = {}


def _hann(n):
    return 0.5 * (1.0 - np.cos(2.0 * np.pi * np.arange(n) / n))


def _build_module(db=DB):
    import concourse.bass as bass
    import concourse.tile as tile
    from concourse import bacc, mybir
    from concourse.ap import AP

    f32 = mybir.dt.float32
    f16 = mybir.dt.float16
    mult = mybir.AluOpType.mult
    add = mybir.AluOpType.add
    copyf = mybir.ActivationFunctionType.Copy

    da = db // 2          # DVE half A blocks [0, da)
    dbb = db - da         # DVE half B blocks [da, db)
    gbn = NBLK - db       # gpsimd blocks [db, NBLK)

    nc = bacc.Bacc("TRN2", target_bir_lowering=False, debug=False)
    exh_in = nc.dram_tensor("exh", [NB, LFULL], f16, kind="ExternalInput").ap()
    # coefficients, block-fastest + section-DESCENDING:
    # col ((NS-s)*2 + pair)*nb + b, pair 0 = c2, 1 = c1
    c21a_in = nc.dram_tensor("c21a", [128, NS * da * 2], f16, kind="ExternalInput").ap()
    c21b_in = nc.dram_tensor("c21b", [128, NS * dbb * 2], f16, kind="ExternalInput").ap()
    c21p_in = nc.dram_tensor("c21p", [128, NS * gbn * 2], f32, kind="ExternalInput").ap()
    gs_in = nc.dram_tensor("gs", [128, NBLK], f32, kind="ExternalInput").ap()
    xs0_in = nc.dram_tensor("xs0", [128, L * NBLK], f16, kind="ExternalInput").ap()
    win_in = nc.dram_tensor("win4", [128, 4], f32, kind="ExternalInput").ap()
    rnt_in = nc.dram_tensor("rnt", [128, 9 * 128], f32, kind="ExternalInput").ap()
    id_in = nc.dram_tensor("idn", [128, 128], f32, kind="ExternalInput").ap()
    idh_in = nc.dram_tensor("idh", [128, 128], f16, kind="ExternalInput").ap()
    out = nc.dram_tensor("out", [NB, T], f32, kind="ExternalOutput").ap()

    with tile.TileContext(nc) as tc:
        with (
            tc.tile_pool(name="state", bufs=1) as st,
            tc.tile_pool(name="scratch", bufs=2) as sp,
            tc.tile_pool(name="psum", bufs=4, space="PSUM") as pp,
        ):
            # lanes: independent instruction streams. DVE lanes A/B are
            # interleaved per wavefront step; Pool runs one lane.
            lanes = []
            for nm, eng, nb_e, b0, dt, cin in (
                    ("a", nc.vector, da, 0, f16, c21a_in),
                    ("b", nc.vector, dbb, da, f16, c21b_in),
                    ("g", nc.gpsimd, gbn, db, f32, c21p_in)):
                H = st.tile([128, nb_e * CW], dt, tag=f"H{nm}", name=f"H{nm}")
                C21 = st.tile([128, NS * nb_e * 2], dt, tag=f"C21{nm}",
                              name=f"C21{nm}")
                lanes.append(dict(nm=nm, eng=eng, nb=nb_e, b0=b0, dt=dt,
                                  H=H, C21=C21, cin=cin))

            Xs = [st.tile([128, NBLK * L], f16, tag=f"Xs{h}", name=f"Xs{h}")
                  for h in range(2)]
            XS0M = st.tile([128, L * NBLK], f16)
            ACC = st.tile([128, NB * ACCW], f16)
            DSB = st.tile([128, 128 * 128], f16)
            GS = st.tile([128, NBLK], f32)
            WIN4 = st.tile([128, 4], f32)
            RNT = st.tile([128, 9 * 128], f32)
            IDN = st.tile([128, 128], f32)
            IDH = st.tile([128, 128], f16)

            # chunk-0 comes pre-unfolded and gain-premultiplied from the
            # host (t-major per lane region); its three per-lane DMAs go
            # first in the HWDGE queue since they gate the wavefront starts.
            xs0t = XS0M[:].tensor
            lane_off = {"a": 0, "b": L * da, "g": L * db}
            TC = 8   # tiny-prefix columns: unblocks wavefront steps 0..TC-1
            # the wavefront needs slot-0 col g+24 at step g, and the t-major
            # layout makes the first TC time-columns a contiguous prefix of
            # each lane's slab: DMA + Act-copy a tiny prefix per DVE lane
            # first so the wavefront starts while the full slabs land.
            for (off, width), e in zip(((0, L * da), (L * da, L * dbb)),
                                       (lanes[0], lanes[1])):
                nb_e = e["nb"]
                nc.sync.dma_start(
                    AP(xs0t, off, [[L * NBLK, 128], [1, TC * nb_e]]),
                    AP(xs0_in.tensor, off, [[L * NBLK, 128], [1, TC * nb_e]]))
                nc.sync.dma_start(e["C21"][:], e["cin"])
                e["eng"].memset(
                    AP(e["H"][:].tensor, 0,
                       [[e["nb"] * CW, 128], [1, 2 * NS * e["nb"]]]), 0.0)
                nc.scalar.copy(
                    AP(e["H"][:].tensor, 24 * nb_e,
                       [[nb_e * CW, 128], [1, TC * nb_e]]),
                    AP(xs0t, off, [[L * NBLK, 128], [1, TC * nb_e]]))
            # gpsimd's full slab (it has schedule slack), then the DVE lanes'
            # remainders
            e = lanes[2]
            nc.sync.dma_start(
                AP(xs0t, L * db, [[L * NBLK, 128], [1, L * gbn]]),
                AP(xs0_in.tensor, L * db, [[L * NBLK, 128], [1, L * gbn]]))
            nc.sync.dma_start(e["C21"][:], e["cin"])
            e["eng"].memset(
                AP(e["H"][:].tensor, 0,
                   [[e["nb"] * CW, 128], [1, 2 * NS * e["nb"]]]), 0.0)
            for (off, width), e in zip(((0, L * da), (L * da, L * dbb)),
                                       (lanes[0], lanes[1])):
                nb_e = e["nb"]
                nc.sync.dma_start(
                    AP(xs0t, off + TC * nb_e,
                       [[L * NBLK, 128], [1, width - TC * nb_e]]),
                    AP(xs0_in.tensor, off + TC * nb_e,
                       [[L * NBLK, 128], [1, width - TC * nb_e]]))
            nc.sync.dma_start(GS[:], gs_in)

            def lane_of(b):
                for e in lanes:
                    if e["b0"] <= b < e["b0"] + e["nb"]:
                        return e
                raise AssertionError

            # input staging: unfold DMA -> Xs, per-block Act copy with
            # per-partition gain scale into the skewed slot-0 columns
            # (cell (0, t) of block b = col (t+24)*nb + b_local).
            # staged lane-first order: gpsimd's lane is the critical path, so
            # its batch row DMAs and Act copies go first each chunk.
            def stage_dma(xst, ct, beta):
                nc.sync.dma_start(
                    AP(xst, beta * 8 * L, [[NBLK * L, 128], [1, 8 * L]]),
                    AP(exh_in.tensor,
                       beta * LFULL + ct * L,
                       [[HOP, 128], [128 * HOP, 8], [1, L]]))

            def stage_act(xst, ct, b):
                e = lane_of(b)
                nb_e = e["nb"]
                bl = b - e["b0"]
                nc.scalar.activation(
                    AP(e["H"][:].tensor, (24 + ct * L) * nb_e + bl,
                       [[nb_e * CW, 128], [nb_e, L]]),
                    AP(xst, b * L, [[NBLK * L, 128], [1, L]]),
                    copyf, scale=GS[:, b:b + 1])

            for ct in range(NCHUNK):
                xs = Xs[ct % 2]
                xst = xs[:].tensor
                if ct > 0:
                    for beta in (0, 1, 2, 3):
                        stage_dma(xst, ct, beta)
                if ct == 0:
                    # one plain Act copy per lane (gain already folded in
                    # on the host); DVE lanes first (remainder after their
                    # tiny prefix), gpsimd's full slab last
                    for e in (lanes[0], lanes[1], lanes[2]):
                        nb_e = e["nb"]
                        tc0 = TC if e["nm"] in ("a", "b") else 0
                        nc.scalar.copy(
                            AP(e["H"][:].tensor, (24 + tc0) * nb_e,
                               [[nb_e * CW, 128], [1, (L - tc0) * nb_e]]),
                            AP(xs0t, lane_off[e["nm"]] + tc0 * nb_e,
                               [[L * NBLK, 128], [1, (L - tc0) * nb_e]]))
                    nc.scalar.memzero(ACC[:])
                    nc.sync.dma_start(WIN4[:], win_in)
                    nc.sync.dma_start(RNT[:], rnt_in)
                    nc.sync.dma_start(IDN[:], id_in)
                    nc.sync.dma_start(IDH[:], idh_in)
                else:
                    for b in list(range(db)) + list(range(db, NBLK)):
                        stage_act(xst, ct, b)

            # wavefront ops for one lane at step g
            def lane_step(e, g):
                nm, eng, nb_e, dt = e["nm"], e["eng"], e["nb"], e["dt"]
                ht = e["H"][:].tensor
                c21t = e["C21"][:].tensor
                hc = nb_e * CW
                s_lo = max(1, g - WIN + 2)
                s_hi = min(NS, g + 1)
                ns = s_hi - s_lo + 1
                w = ns * nb_e
                off_w = g - 3 * s_hi + 25
                pr = sp.tile([128, NS * nb_e * 2], dt, tag=f"pr{nm}",
                             name=f"pr{nm}")
                t2 = sp.tile([128, NS * nb_e], dt, tag=f"t2{nm}",
                             name=f"t2{nm}")
                prt = pr[:].tensor
                t2t = t2[:].tensor
                pra = AP(prt, 0, [[NS * nb_e * 2, 128], [2 * nb_e, ns],
                                  [nb_e, 2], [1, nb_e]])
                h_pair = AP(ht, (off_w - 2) * nb_e,
                            [[hc, 128], [3 * nb_e, ns], [nb_e, 2], [1, nb_e]])
                c_pair = AP(c21t, (NS - s_hi) * 2 * nb_e,
                            [[NS * nb_e * 2, 128], [2 * nb_e, ns],
                             [nb_e, 2], [1, nb_e]])
                t2a = AP(t2t, 0, [[NS * nb_e, 128], [nb_e, ns], [1, nb_e]])
                pr_ev = AP(prt, 0, [[NS * nb_e * 2, 128], [2 * nb_e, ns],
                                    [1, nb_e]])
                pr_od = AP(prt, nb_e, [[NS * nb_e * 2, 128], [2 * nb_e, ns],
                                       [1, nb_e]])
                h_out = AP(ht, off_w * nb_e,
                           [[hc, 128], [3 * nb_e, ns], [1, nb_e]])
                h_x = AP(ht, (off_w + 2) * nb_e,
                         [[hc, 128], [3 * nb_e, ns], [1, nb_e]])
                if dt == f16:
                    yield lambda: eng.tensor_tensor(pra, h_pair, c_pair, op=mult)
                    yield lambda: eng.tensor_tensor(t2a, pr_ev, pr_od, op=add)
                    yield lambda: eng.tensor_tensor(h_out, t2a, h_x, op=add)
                else:
                    yield lambda: eng.tensor_tensor(pra, h_pair, c_pair, op=mult)
                    yield lambda: eng.tensor_tensor(t2a, pr_ev, pr_od, op=add)
                    yield lambda: eng.tensor_tensor(h_out, t2a, h_x, op=add)

            # merged wavefront loop: DVE lanes A/B interleaved per step (hides
            # the same-engine sem latency), Pool lane serial.
            for g in range(WIN + NS - 1):
                for opa, opb in zip(lane_step(lanes[0], g),
                                    lane_step(lanes[1], g)):
                    opa()
                    opb()
                for op in lane_step(lanes[2], g):
                    op()

            # overlap-add epilogue: transpose each block's 128 final (section
            # 11) time-columns to PSUM, then stt window+accumulate into ACC.
            # Batch-row rotation interleaves the per-row ACC RMW chains.
            # The DVE-half epilogue is emitted first; a no_sync scheduler
            # barrier then keeps the Pool-half transposes (which become ready
            # only at Pool wavefront end) BEHIND the DVE-half ones in PE's
            # in-order stream, so they can't head-of-line-block them.
            def rotate(blks):
                # round-robin across batch rows so consecutive adds hit
                # disjoint ACC ranges (the per-row RMW chains interleave)
                groups = {}
                for b in blks:
                    groups.setdefault(b // 8, []).append(b)
                order = []
                gs = list(groups.values())
                i = 0
                while any(gs):
                    for g in gs:
                        if i < len(g):
                            order.append(g[i])
                    i += 1
                    if all(i >= len(g) for g in gs):
                        break
                return order

            def epi_stage(j, blks, pstag, bufs):
                # PE transpose each block's window-j columns to PSUM; Act
                # folds the hann window into the PSUM->SBUF copy
                # (per-partition scale = win chunk j), pre-staged into a
                # dedicated SBUF slice so the DVE adds can run back-to-back.
                for b in blks:
                    e = lane_of(b)
                    bl = b - e["b0"]
                    nb_e = e["nb"]
                    ps = pp.tile([128, 128], e["dt"], tag=pstag, name="ps",
                                 bufs=bufs)
                    nc.tensor.transpose(
                        ps[:],
                        AP(e["H"][:].tensor, (2 + j * 128) * nb_e + bl,
                           [[nb_e * CW, 128], [nb_e, 128]]),
                        IDH[:] if e["dt"] == f16 else IDN[:])
                    sl = (j * 32 + b) * 128
                    nc.scalar.activation(DSB[:, sl:sl + 128], ps[:],
                                         copyf, scale=WIN4[:, j:j + 1])

            def epi_add(j, blks):
                for b in rotate(blks):
                    beta, bb = divmod(b, 8)
                    k0 = beta * ACCW + bb * 128 + j
                    sl = (j * 32 + b) * 128
                    nc.vector.tensor_tensor(
                        ACC[:, k0:k0 + 128], DSB[:, sl:sl + 128],
                        ACC[:, k0:k0 + 128], op=add)

            # output pass for one batch row: transpose ACC back to
            # sample-major, multiply 1/norm into a per-row staging buffer,
            # then drain with FOUR DMAs (the 7 middle chunks are contiguous
            # in the output, so one strided DMA covers them; SP.SEQ DMA
            # dispatch is 650ns each and 9 per row would serialize the tail).
            def output_beta(beta):
                otb = st.tile([128, 9 * 128], f32, tag=f"otb{beta}",
                              name=f"otb{beta}")
                o_t = otb[:].tensor
                for i, k0 in enumerate(K0S):
                    ps = pp.tile([128, 128], f16, tag="pso", name="pso",
                                 bufs=3)
                    nc.tensor.transpose(
                        ps[:], ACC[:, beta * ACCW + k0:beta * ACCW + k0 + 128],
                        IDH[:])
                    if i in (1, 3, 5):
                        # interior chunks (norm folded): alternate the plain
                        # copies between Act and DVE so neither serializes
                        # the pso ring pipeline in the tail
                        nc.scalar.copy(otb[:, i * 128:(i + 1) * 128], ps[:])
                    else:
                        nc.vector.tensor_tensor(
                            otb[:, i * 128:(i + 1) * 128], ps[:],
                            RNT[:, i * 128:(i + 1) * 128], op=mult)
                    if i == 0:
                        nc.sync.dma_start(
                            AP(out.tensor, beta * T, [[1, 1], [1, 64]]),
                            AP(o_t, 64, [[9 * 128, 1], [1, 64]]))
                        nc.sync.dma_start(
                            AP(out.tensor, beta * T + 64, [[128, 127], [1, 128]]),
                            AP(o_t, 9 * 128, [[9 * 128, 127], [1, 128]]))
                    elif i == 7:
                        nc.sync.dma_start(
                            AP(out.tensor, beta * T + 129 * 128 - PAD,
                               [[128, 128], [128 * 128, 7], [1, 128]]),
                            AP(o_t, 128, [[9 * 128, 128], [128, 7], [1, 128]]))
                    elif i == 8:
                        nc.sync.dma_start(
                            AP(out.tensor, beta * T + 1025 * 128 - PAD,
                               [[1, 1], [1, 64]]),
                            AP(o_t, 8 * 128 + 127 * 9 * 128, [[9 * 128, 1], [1, 64]]))

            # DVE-half epilogue first; Pool-half (which becomes ready only
            # at Pool wavefront end) stays behind a scheduler barrier so it
            # can't head-of-line block DVE-dependent PE work.
            gblks = list(range(db, NBLK))
            for j in range(4):
                epi_stage(j, list(range(db)), "psD", 3)
                if j < 3:
                    # the Pool-half j0-j2 windows are final mid-wavefront;
                    # staging them here (PE+Act, both idle) leaves only the
                    # j3 chunk for the post-Pool tail
                    epi_stage(j, gblks, "psG", 2)
                epi_add(j, list(range(db)))
                if j < 3:
                    epi_add(j, gblks)
            # batch rows fully covered by DVE-lane blocks are complete;
            # their output overlaps the Pool-half epilogue below.
            for beta in range(db // 8):
                output_beta(beta)
            tc.no_sync_barrier()
            epi_stage(3, gblks, "psG", 2)
            epi_add(3, gblks)
            tc.no_sync_barrier()
            for beta in range(db // 8, NB):
                output_beta(beta)

    nc.compile()
    return nc


def _host_prep(ex, gain, biquads, db=DB):
    # per-core host tiles; frame n of batch row beta: p = n % 128, j = n//128,
    # block b = beta*8 + j
    f32, f16 = np.float32, np.float16
    da = db // 2
    a0 = biquads[..., 0].astype(f32)
    a1 = biquads[..., 1].astype(f32)
    a2 = biquads[..., 2].astype(f32)
    c1 = (-a1 / a0).astype(f32)          # [NB, F, S]
    c2 = (-a2 / a0).astype(f32)
    gain_eff = (gain.astype(f32) * np.prod((1.0 / a0).astype(f32), axis=-1)).astype(f32)

    c1r = c1.reshape(NB, 8, 128, S).transpose(2, 3, 0, 1).reshape(128, S, NBLK)
    c2r = c2.reshape(NB, 8, 128, S).transpose(2, 3, 0, 1).reshape(128, S, NBLK)
    C = np.stack([c2r, c1r], axis=2)[:, ::-1]     # [128, S(desc), 2, NBLK]
    C21a = np.ascontiguousarray(
        C[..., :da].reshape(128, S * 2 * da)).astype(f16)
    C21b = np.ascontiguousarray(
        C[..., da:db].reshape(128, S * 2 * (db - da))).astype(f16)
    C21p = np.ascontiguousarray(
        C[..., db:].reshape(128, S * 2 * (NBLK - db))).astype(f32)
    GSm = gain_eff.reshape(NB, 8, 128).transpose(2, 0, 1).reshape(128, NBLK)
    GSm = np.ascontiguousarray(GSm).astype(f32)
    expd = np.pad(ex.astype(f32), ((0, 0), (PAD, PAD)))
    # chunk-0 pre-unfolded + gain-premultiplied, t-major per lane region:
    # region(lane) + t*nb + bl = gain[p, b] * expd[beta, frame*HOP + t]
    arr = np.empty((128, NBLK, L), f32)
    pp_ = np.arange(128)[:, None]
    tt_ = np.arange(L)[None, :]
    for b in range(NBLK):
        arr[:, b, :] = expd[b // 8][((b % 8) * 128 + pp_) * HOP + tt_]
    arr *= GSm[:, :, None]
    xs0 = np.concatenate(
        [arr[:, lo:hi].transpose(0, 2, 1).reshape(128, -1)
         for lo, hi in ((0, da), (da, db), (db, NBLK))], axis=1)
    xs0 = np.ascontiguousarray(xs0).astype(f16)
    exh = expd.astype(f16)
    return exh, C21a, C21b, C21p, GSm, xs0


def _host_consts():
    f32 = np.float32
    win = _hann(WIN).astype(f32)
    WIN4 = np.ascontiguousarray(win.reshape(4, 128).T) * f32(0.5)
    norm = np.zeros(LFULL, f32)
    idx = (np.arange(F)[:, None] * HOP + np.arange(WIN)[None, :]).reshape(-1)
    np.add.at(norm, idx, np.broadcast_to(win, (F, WIN)).reshape(-1))
    rn = np.zeros_like(norm)
    nz = norm != 0
    rn[nz] = (f32(2.0) / norm[nz]).astype(f32)
    rn2 = rn.reshape(NCELL, 128)         # [k, p]
    RNT = np.zeros((128, 9 * 128), f32)  # [k_local, i*128 + p]
    for i, k0 in enumerate(K0S):
        RNT[:, i * 128:(i + 1) * 128] = rn2[k0:k0 + 128, :]
    IDN = np.eye(128, dtype=f32)
    return WIN4, RNT, IDN, IDN.astype(np.float16)


def _build_in_maps(ex, gain, biquads):
    WIN4, RNT, IDN, IDH = _host_consts()
    in_maps = []
    for ci in range(NCORE):
        sl = slice(ci * NB, (ci + 1) * NB)
        exh, C21a, C21b, C21p, GSm, xs0 = _host_prep(
            ex[sl], gain[sl], biquads[sl])
        in_maps.append({
            "exh": exh, "c21a": C21a, "c21b": C21b, "c21p": C21p, "gs": GSm,
            "xs0": xs0,
            "win4": WIN4, "rnt": RNT, "idn": IDN, "idh": IDH,
        })
    return in_maps


def kernel(ex, gain, biquads):
    from concourse.bass_utils import run_bass_kernel_spmd

    ex = np.asarray(ex, np.float32)
    gain = np.asarray(gain, np.float32)
    biquads = np.asarray(biquads, np.float32)

    if "nc" not in _CACHE:
        _CACHE["nc"] = _build_module()
    nc = _CACHE["nc"]

    in_maps = _build_in_maps(ex, gain, biquads)
    res = run_bass_kernel_spmd(nc, in_maps, list(range(NCORE)))
    out = np.concatenate([res.results[ci]["out"] for ci in range(NCORE)], axis=0)
    return out.astype(np.float32)


# revision 58
# speedup vs baseline: 1.0010x; 1.0007x over previous
# Trainium2 Bass kernel: batched second-order LPC synthesis
# (frame unfold -> gain -> 11 cascaded biquads -> hann window -> overlap-add -> norm)
#
# Sharding: pure data parallel over batch. 32 batch rows / 8 cores = 4 rows per
# core; each core handles 4*1024 = 4096 frames laid out as 128 partitions x 32
# frame-blocks.
#
# Design (driven by the TimelineSim cost model + HW legality):
#  - the 11-section biquad cascade runs as a wavefront over (section, time):
#    wavefront step g updates section s at local time t = g-s+1 for all frames
#    at once with 3 elementwise ops (pair products, pair add, add-x).
#  - state is SKEWED and BLOCK-FASTEST: cell (s, t) of frame-block b sits at
#    column (t - 2s + 24)*nb + b of its lane's state tile. Block index is the
#    innermost (stride-1) AP dim of every wavefront operand, so with fp16
#    operands the DVE runs its 2x_1p packed mode (0.52 ns/elem vs 1.04).
#  - the Tile framework chains same-engine instructions through semaphores;
#    dependent back-to-back instructions pay a ~95ns (DVE) / ~62ns (Pool)
#    ack+propagate bubble. The DVE wavefront is therefore split into TWO
#    independent interleaved half-wavefronts (A/B block halves, separate
#    tiles): each instruction's producer is 2 back, hiding the latency.
#  - DVE (fp16, tensor_tensor 2x) takes DB=26 blocks as two halves; gpsimd
#    (fp32 tensor_tensor; scalar_tensor_tensor is NOT a legal Pool opcode on
#    HW, and gpsimd cannot touch PSUM) runs the remaining 6 blocks as one
#    serial wavefront (its per-instruction Q7 launch overhead outweighs the
#    smaller sem bubble, so no interleave).
#  - input staging: host-padded fp16 excitation is DMA-unfolded into a
#    double-buffered Xs tile; the gain multiply into the skewed slot-0
#    columns runs on the otherwise-idle Act engine (func=Copy,
#    scale=per-partition gain column). Chunk 0 arrives pre-unfolded and
#    gain-premultiplied from the host (t-major per lane region, DMAs issued
#    ahead of all constant loads) and is placed by one plain Act copy per
#    lane, with a tiny 8-column prefix DMA+copy per DVE lane first (the
#    wavefront needs slot-0 col g+24 at step g), so the wavefronts start
#    ~3us in while the full slabs land.
#  - output DMAs are batched: the 7 middle chunks of each batch row are
#    contiguous in the output, so one strided DMA drains them (SP.SEQ DMA
#    dispatch costs 650ns each; 9 per row serialized the tail).
#  - window + overlap-add epilogue: PE transposes each block's 128
#    final-section time-columns to PSUM (fp16 1 cyc/row); the Act engine
#    folds the hann window into its PSUM->SBUF copy (scale=win chunk) into a
#    dedicated staging buffer (DSB); the accumulate is then a packed-fp16 2x
#    tensor_tensor add into the fp16 ACC (DVE; batch row 3 on gpsimd, which
#    is idle by then). Adds rotate across batch rows so the per-row ACC RMW
#    chains interleave without sem bubbles. A no_sync scheduler barrier keeps
#    the Pool-half transposes (ready only at Pool wavefront end) BEHIND the
#    DVE-half ones in PE's in-order stream, so they can't head-of-line block
#    them (the Tile scheduler otherwise orders PE's stream by its own
#    optimistic Pool timing).
#  - output: PE transposes the fp16 accumulator back to sample-major. The
#    OLA norm of a 75%-overlap hann is exactly 2 away from the edges, and
#    that 1/2 is folded into the window scale, so the 6 interior chunks per
#    batch row are plain Act PSUM->SBUF copies; only the edge chunks need a
#    DVE multiply by 2/norm. Contiguous-row DMAs write the cropped fp32
#    result; batch rows 0-2 are emitted before the Pool-half epilogue so
#    they overlap it.
import numpy as np

HOP, WIN, PAD = 128, 512, 192
B, T, S = 32, 131072, 11
F = T // HOP          # 1024
NCORE = 8
NB = B // NCORE       # 4 batch rows per core
L = 128               # input staging chunk (columns of a frame)
NCHUNK = WIN // L     # 4
NBLK = (NB * F) // 128  # 32 frame blocks per core
NS = 11
CW = WIN + 2 * NS + 2   # 536 skewed column-groups per block
LFULL = T + 2 * PAD   # 131456
NCELL = LFULL // HOP  # 1027
ACCW = 1028
DB = 26               # frame blocks on DVE (two interleaved halves)
K0S = [1 + 128 * i for i in range(8)] + [898]  # output transpose col bases

_CACHE = {}


def _hann(n):
    return 0.5 * (1.0 - np.cos(2.0 * np.pi * np.arange(n) / n))


def _build_module(db=DB):
    import concourse.bass as bass
    import concourse.tile as tile
    from concourse import bacc, mybir
    from concourse.ap import AP

    f32 = mybir.dt.float32
    f16 = mybir.dt.float16
    mult = mybir.AluOpType.mult
    add = mybir.AluOpType.add
    copyf = mybir.ActivationFunctionType.Copy

    da = db // 2          # DVE half A blocks [0, da)
    dbb = db - da         # DVE half B blocks [da, db)
    gbn = NBLK - db       # gpsimd blocks [db, NBLK)

    nc = bacc.Bacc("TRN2", target_bir_lowering=False, debug=False)
    exh_in = nc.dram_tensor("exh", [NB, LFULL], f16, kind="ExternalInput").ap()
    # coefficients, block-fastest + section-DESCENDING:
    # col ((NS-s)*2 + pair)*nb + b, pair 0 = c2, 1 = c1
    c21a_in = nc.dram_tensor("c21a", [128, NS * da * 2], f16, kind="ExternalInput").ap()
    c21b_in = nc.dram_tensor("c21b", [128, NS * dbb * 2], f16, kind="ExternalInput").ap()
    c21p_in = nc.dram_tensor("c21p", [128, NS * gbn * 2], f32, kind="ExternalInput").ap()
    gs_in = nc.dram_tensor("gs", [128, NBLK], f32, kind="ExternalInput").ap()
    xs0_in = nc.dram_tensor("xs0", [128, L * NBLK], f16, kind="ExternalInput").ap()
    win_in = nc.dram_tensor("win4", [128, 4], f32, kind="ExternalInput").ap()
    rnt_in = nc.dram_tensor("rnt", [128, 9 * 128], f32, kind="ExternalInput").ap()
    id_in = nc.dram_tensor("idn", [128, 128], f32, kind="ExternalInput").ap()
    idh_in = nc.dram_tensor("idh", [128, 128], f16, kind="ExternalInput").ap()
    out = nc.dram_tensor("out", [NB, T], f32, kind="ExternalOutput").ap()

    with tile.TileContext(nc) as tc:
        with (
            tc.tile_pool(name="state", bufs=1) as st,
            tc.tile_pool(name="scratch", bufs=2) as sp,
            tc.tile_pool(name="psum", bufs=4, space="PSUM") as pp,
        ):
            # lanes: independent instruction streams. DVE lanes A/B are
            # interleaved per wavefront step; Pool runs one lane.
            lanes = []
            for nm, eng, nb_e, b0, dt, cin in (
                    ("a", nc.vector, da, 0, f16, c21a_in),
                    ("b", nc.vector, dbb, da, f16, c21b_in),
                    ("g", nc.gpsimd, gbn, db, f32, c21p_in)):
                H = st.tile([128, nb_e * CW], dt, tag=f"H{nm}", name=f"H{nm}")
                C21 = st.tile([128, NS * nb_e * 2], dt, tag=f"C21{nm}",
                              name=f"C21{nm}")
                lanes.append(dict(nm=nm, eng=eng, nb=nb_e, b0=b0, dt=dt,
                                  H=H, C21=C21, cin=cin))

            Xs = [st.tile([128, NBLK * L], f16, tag=f"Xs{h}", name=f"Xs{h}")
                  for h in range(2)]
            XS0M = st.tile([128, L * NBLK], f16)
            ACC = st.tile([128, NB * ACCW], f16)
            DSB = st.tile([128, 128 * 128], f16)
            GS = st.tile([128, NBLK], f32)
            WIN4 = st.tile([128, 4], f32)
            RNT = st.tile([128, 9 * 128], f32)
            IDN = st.tile([128, 128], f32)
            IDH = st.tile([128, 128], f16)

            # chunk-0 comes pre-unfolded and gain-premultiplied from the
            # host (t-major per lane region); its three per-lane DMAs go
            # first in the HWDGE queue since they gate the wavefront starts.
            xs0t = XS0M[:].tensor
            lane_off = {"a": 0, "b": L * da, "g": L * db}
            TC = 8   # tiny-prefix columns: unblocks wavefront steps 0..TC-1
            # the wavefront needs slot-0 col g+24 at step g, and the t-major
            # layout makes the first TC time-columns a contiguous prefix of
            # each lane's slab: DMA + Act-copy a tiny prefix per DVE lane
            # first so the wavefront starts while the full slabs land.
            for (off, width), e in zip(((0, L * da), (L * da, L * dbb)),
                                       (lanes[0], lanes[1])):
                nb_e = e["nb"]
                nc.sync.dma_start(
                    AP(xs0t, off, [[L * NBLK, 128], [1, TC * nb_e]]),
                    AP(xs0_in.tensor, off, [[L * NBLK, 128], [1, TC * nb_e]]))
                nc.sync.dma_start(e["C21"][:], e["cin"])
                e["eng"].memset(
                    AP(e["H"][:].tensor, 0,
                       [[e["nb"] * CW, 128], [1, 2 * NS * e["nb"]]]), 0.0)
                nc.scalar.copy(
                    AP(e["H"][:].tensor, 24 * nb_e,
                       [[nb_e * CW, 128], [1, TC * nb_e]]),
                    AP(xs0t, off, [[L * NBLK, 128], [1, TC * nb_e]]))
            # gpsimd's full slab (it has schedule slack), then the DVE lanes'
            # remainders
            e = lanes[2]
            nc.sync.dma_start(
                AP(xs0t, L * db, [[L * NBLK, 128], [1, L * gbn]]),
                AP(xs0_in.tensor, L * db, [[L * NBLK, 128], [1, L * gbn]]))
            nc.sync.dma_start(e["C21"][:], e["cin"])
            e["eng"].memset(
                AP(e["H"][:].tensor, 0,
                   [[e["nb"] * CW, 128], [1, 2 * NS * e["nb"]]]), 0.0)
            for (off, width), e in zip(((0, L * da), (L * da, L * dbb)),
                                       (lanes[0], lanes[1])):
                nb_e = e["nb"]
                nc.sync.dma_start(
                    AP(xs0t, off + TC * nb_e,
                       [[L * NBLK, 128], [1, width - TC * nb_e]]),
                    AP(xs0_in.tensor, off + TC * nb_e,
                       [[L * NBLK, 128], [1, width - TC * nb_e]]))
            nc.sync.dma_start(GS[:], gs_in)

            def lane_of(b):
                for e in lanes:
                    if e["b0"] <= b < e["b0"] + e["nb"]:
                        return e
                raise AssertionError

            # input staging: unfold DMA -> Xs, per-block Act copy with
            # per-partition gain scale into the skewed slot-0 columns
            # (cell (0, t) of block b = col (t+24)*nb + b_local).
            # staged lane-first order: gpsimd's lane is the critical path, so
            # its batch row DMAs and Act copies go first each chunk.
            def stage_dma(xst, ct, beta):
                nc.sync.dma_start(
                    AP(xst, beta * 8 * L, [[NBLK * L, 128], [1, 8 * L]]),
                    AP(exh_in.tensor,
                       beta * LFULL + ct * L,
                       [[HOP, 128], [128 * HOP, 8], [1, L]]))

            def stage_act(xst, ct, b):
                e = lane_of(b)
                nb_e = e["nb"]
                bl = b - e["b0"]
                nc.scalar.activation(
                    AP(e["H"][:].tensor, (24 + ct * L) * nb_e + bl,
                       [[nb_e * CW, 128], [nb_e, L]]),
                    AP(xst, b * L, [[NBLK * L, 128], [1, L]]),
                    copyf, scale=GS[:, b:b + 1])

            for ct in range(NCHUNK):
                xs = Xs[ct % 2]
                xst = xs[:].tensor
                if ct > 0:
                    for beta in (0, 1, 2, 3):
                        stage_dma(xst, ct, beta)
                if ct == 0:
                    # one plain Act copy per lane (gain already folded in
                    # on the host); DVE lanes first (remainder after their
                    # tiny prefix), gpsimd's full slab last
                    for e in (lanes[0], lanes[1], lanes[2]):
                        nb_e = e["nb"]
                        tc0 = TC if e["nm"] in ("a", "b") else 0
                        nc.scalar.copy(
                            AP(e["H"][:].tensor, (24 + tc0) * nb_e,
                               [[nb_e * CW, 128], [1, (L - tc0) * nb_e]]),
                            AP(xs0t, lane_off[e["nm"]] + tc0 * nb_e,
                               [[L * NBLK, 128], [1, (L - tc0) * nb_e]]))
                    nc.scalar.memzero(ACC[:])
                    nc.sync.dma_start(WIN4[:], win_in)
                    nc.sync.dma_start(RNT[:], rnt_in)
                    nc.sync.dma_start(IDN[:], id_in)
                    nc.sync.dma_start(IDH[:], idh_in)
                else:
                    for b in list(range(db)) + list(range(db, NBLK)):
                        stage_act(xst, ct, b)

            # wavefront ops for one lane at step g
            def lane_step(e, g):
                nm, eng, nb_e, dt = e["nm"], e["eng"], e["nb"], e["dt"]
                ht = e["H"][:].tensor
                c21t = e["C21"][:].tensor
                hc = nb_e * CW
                s_lo = max(1, g - WIN + 2)
                s_hi = min(NS, g + 1)
                ns = s_hi - s_lo + 1
                w = ns * nb_e
                off_w = g - 3 * s_hi + 25
                pr = sp.tile([128, NS * nb_e * 2], dt, tag=f"pr{nm}",
                             name=f"pr{nm}")
                t2 = sp.tile([128, NS * nb_e], dt, tag=f"t2{nm}",
                             name=f"t2{nm}")
                prt = pr[:].tensor
                t2t = t2[:].tensor
                pra = AP(prt, 0, [[NS * nb_e * 2, 128], [2 * nb_e, ns],
                                  [nb_e, 2], [1, nb_e]])
                h_pair = AP(ht, (off_w - 2) * nb_e,
                            [[hc, 128], [3 * nb_e, ns], [nb_e, 2], [1, nb_e]])
                c_pair = AP(c21t, (NS - s_hi) * 2 * nb_e,
                            [[NS * nb_e * 2, 128], [2 * nb_e, ns],
                             [nb_e, 2], [1, nb_e]])
                t2a = AP(t2t, 0, [[NS * nb_e, 128], [nb_e, ns], [1, nb_e]])
                pr_ev = AP(prt, 0, [[NS * nb_e * 2, 128], [2 * nb_e, ns],
                                    [1, nb_e]])
                pr_od = AP(prt, nb_e, [[NS * nb_e * 2, 128], [2 * nb_e, ns],
                                       [1, nb_e]])
                h_out = AP(ht, off_w * nb_e,
                           [[hc, 128], [3 * nb_e, ns], [1, nb_e]])
                h_x = AP(ht, (off_w + 2) * nb_e,
                         [[hc, 128], [3 * nb_e, ns], [1, nb_e]])
                if dt == f16:
                    yield lambda: eng.tensor_tensor(pra, h_pair, c_pair, op=mult)
                    yield lambda: eng.tensor_tensor(t2a, pr_ev, pr_od, op=add)
                    yield lambda: eng.tensor_tensor(h_out, t2a, h_x, op=add)
                else:
                    yield lambda: eng.tensor_tensor(pra, h_pair, c_pair, op=mult)
                    yield lambda: eng.tensor_tensor(t2a, pr_ev, pr_od, op=add)
                    yield lambda: eng.tensor_tensor(h_out, t2a, h_x, op=add)

            # merged wavefront loop: DVE lanes A/B interleaved per step (hides
            # the same-engine sem latency), Pool lane serial.
            for g in range(WIN + NS - 1):
                for opa, opb in zip(lane_step(lanes[0], g),
                                    lane_step(lanes[1], g)):
                    opa()
                    opb()
                for op in lane_step(lanes[2], g):
                    op()

            # overlap-add epilogue: transpose each block's 128 final (section
            # 11) time-columns to PSUM, then stt window+accumulate into ACC.
            # Batch-row rotation interleaves the per-row ACC RMW chains.
            # The DVE-half epilogue is emitted first; a no_sync scheduler
            # barrier then keeps the Pool-half transposes (which become ready
            # only at Pool wavefront end) BEHIND the DVE-half ones in PE's
            # in-order stream, so they can't head-of-line-block them.
            def rotate(blks):
                # round-robin across batch rows so consecutive adds hit
                # disjoint ACC ranges (the per-row RMW chains interleave)
                groups = {}
                for b in blks:
                    groups.setdefault(b // 8, []).append(b)
                order = []
                gs = list(groups.values())
                i = 0
                while any(gs):
                    for g in gs:
                        if i < len(g):
                            order.append(g[i])
                    i += 1
                    if all(i >= len(g) for g in gs):
                        break
                return order

            def epi_stage(j, blks, pstag, bufs):
                # PE transpose each block's window-j columns to PSUM; Act
                # folds the hann window into the PSUM->SBUF copy
                # (per-partition scale = win chunk j), pre-staged into a
                # dedicated SBUF slice so the DVE adds can run back-to-back.
                for b in blks:
                    e = lane_of(b)
                    bl = b - e["b0"]
                    nb_e = e["nb"]
                    ps = pp.tile([128, 128], e["dt"], tag=pstag, name="ps",
                                 bufs=bufs)
                    nc.tensor.transpose(
                        ps[:],
                        AP(e["H"][:].tensor, (2 + j * 128) * nb_e + bl,
                           [[nb_e * CW, 128], [nb_e, 128]]),
                        IDH[:] if e["dt"] == f16 else IDN[:])
                    sl = (j * 32 + b) * 128
                    nc.scalar.activation(DSB[:, sl:sl + 128], ps[:],
                                         copyf, scale=WIN4[:, j:j + 1])

            def epi_add(j, blks):
                for b in rotate(blks):
                    beta, bb = divmod(b, 8)
                    k0 = beta * ACCW + bb * 128 + j
                    sl = (j * 32 + b) * 128
                    nc.vector.tensor_tensor(
                        ACC[:, k0:k0 + 128], DSB[:, sl:sl + 128],
                        ACC[:, k0:k0 + 128], op=add)

            # output pass for one batch row: transpose ACC back to
            # sample-major, multiply 1/norm into a per-row staging buffer,
            # then drain with FOUR DMAs (the 7 middle chunks are contiguous
            # in the output, so one strided DMA covers them; SP.SEQ DMA
            # dispatch is 650ns each and 9 per row would serialize the tail).
            def output_beta(beta):
                otb = st.tile([128, 9 * 128], f32, tag=f"otb{beta}",
                              name=f"otb{beta}")
                o_t = otb[:].tensor
                for i, k0 in enumerate(K0S):
                    ps = pp.tile([128, 128], f16, tag="pso", name="pso",
                                 bufs=4)
                    nc.tensor.transpose(
                        ps[:], ACC[:, beta * ACCW + k0:beta * ACCW + k0 + 128],
                        IDH[:])
                    if i in (1, 3, 5):
                        # interior chunks (norm folded): alternate the plain
                        # copies between Act and DVE so neither serializes
                        # the pso ring pipeline in the tail
                        nc.scalar.copy(otb[:, i * 128:(i + 1) * 128], ps[:])
                    else:
                        nc.vector.tensor_tensor(
                            otb[:, i * 128:(i + 1) * 128], ps[:],
                            RNT[:, i * 128:(i + 1) * 128], op=mult)
                    if i == 0:
                        nc.sync.dma_start(
                            AP(out.tensor, beta * T, [[1, 1], [1, 64]]),
                            AP(o_t, 64, [[9 * 128, 1], [1, 64]]))
                        nc.sync.dma_start(
                            AP(out.tensor, beta * T + 64, [[128, 127], [1, 128]]),
                            AP(o_t, 9 * 128, [[9 * 128, 127], [1, 128]]))
                    elif i == 7:
                        nc.sync.dma_start(
                            AP(out.tensor, beta * T + 129 * 128 - PAD,
                               [[128, 128], [128 * 128, 7], [1, 128]]),
                            AP(o_t, 128, [[9 * 128, 128], [128, 7], [1, 128]]))
                    elif i == 8:
                        nc.sync.dma_start(
                            AP(out.tensor, beta * T + 1025 * 128 - PAD,
                               [[1, 1], [1, 64]]),
                            AP(o_t, 8 * 128 + 127 * 9 * 128, [[9 * 128, 1], [1, 64]]))

            # DVE-half epilogue first; Pool-half (which becomes ready only
            # at Pool wavefront end) stays behind a scheduler barrier so it
            # can't head-of-line block DVE-dependent PE work.
            gblks = list(range(db, NBLK))
            for j in range(4):
                epi_stage(j, list(range(db)), "psD", 2)
                if j < 3:
                    # the Pool-half j0-j2 windows are final mid-wavefront;
                    # staging them here (PE+Act, both idle) leaves only the
                    # j3 chunk for the post-Pool tail
                    epi_stage(j, gblks, "psG", 2)
                epi_add(j, list(range(db)))
                if j < 3:
                    epi_add(j, gblks)
            # batch rows fully covered by DVE-lane blocks are complete;
            # their output overlaps the Pool-half epilogue below.
            for beta in range(db // 8):
                output_beta(beta)
            tc.no_sync_barrier()
            epi_stage(3, gblks, "psG", 2)
            epi_add(3, gblks)
            tc.no_sync_barrier()
            for beta in range(db // 8, NB):
                output_beta(beta)

    nc.compile()
    return nc


def _host_prep(ex, gain, biquads, db=DB):
    # per-core host tiles; frame n of batch row beta: p = n % 128, j = n//128,
    # block b = beta*8 + j
    f32, f16 = np.float32, np.float16
    da = db // 2
    a0 = biquads[..., 0].astype(f32)
    a1 = biquads[..., 1].astype(f32)
    a2 = biquads[..., 2].astype(f32)
    c1 = (-a1 / a0).astype(f32)          # [NB, F, S]
    c2 = (-a2 / a0).astype(f32)
    gain_eff = (gain.astype(f32) * np.prod((1.0 / a0).astype(f32), axis=-1)).astype(f32)

    c1r = c1.reshape(NB, 8, 128, S).transpose(2, 3, 0, 1).reshape(128, S, NBLK)
    c2r = c2.reshape(NB, 8, 128, S).transpose(2, 3, 0, 1).reshape(128, S, NBLK)
    C = np.stack([c2r, c1r], axis=2)[:, ::-1]     # [128, S(desc), 2, NBLK]
    C21a = np.ascontiguousarray(
        C[..., :da].reshape(128, S * 2 * da)).astype(f16)
    C21b = np.ascontiguousarray(
        C[..., da:db].reshape(128, S * 2 * (db - da))).astype(f16)
    C21p = np.ascontiguousarray(
        C[..., db:].reshape(128, S * 2 * (NBLK - db))).astype(f32)
    GSm = gain_eff.reshape(NB, 8, 128).transpose(2, 0, 1).reshape(128, NBLK)
    GSm = np.ascontiguousarray(GSm).astype(f32)
    expd = np.pad(ex.astype(f32), ((0, 0), (PAD, PAD)))
    # chunk-0 pre-unfolded + gain-premultiplied, t-major per lane region:
    # region(lane) + t*nb + bl = gain[p, b] * expd[beta, frame*HOP + t]
    arr = np.empty((128, NBLK, L), f32)
    pp_ = np.arange(128)[:, None]
    tt_ = np.arange(L)[None, :]
    for b in range(NBLK):
        arr[:, b, :] = expd[b // 8][((b % 8) * 128 + pp_) * HOP + tt_]
    arr *= GSm[:, :, None]
    xs0 = np.concatenate(
        [arr[:, lo:hi].transpose(0, 2, 1).reshape(128, -1)
         for lo, hi in ((0, da), (da, db), (db, NBLK))], axis=1)
    xs0 = np.ascontiguousarray(xs0).astype(f16)
    exh = expd.astype(f16)
    return exh, C21a, C21b, C21p, GSm, xs0


def _host_consts():
    f32 = np.float32
    win = _hann(WIN).astype(f32)
    WIN4 = np.ascontiguousarray(win.reshape(4, 128).T) * f32(0.5)
    norm = np.zeros(LFULL, f32)
    idx = (np.arange(F)[:, None] * HOP + np.arange(WIN)[None, :]).reshape(-1)
    np.add.at(norm, idx, np.broadcast_to(win, (F, WIN)).reshape(-1))
    rn = np.zeros_like(norm)
    nz = norm != 0
    rn[nz] = (f32(2.0) / norm[nz]).astype(f32)
    rn2 = rn.reshape(NCELL, 128)         # [k, p]
    RNT = np.zeros((128, 9 * 128), f32)  # [k_local, i*128 + p]
    for i, k0 in enumerate(K0S):
        RNT[:, i * 128:(i + 1) * 128] = rn2[k0:k0 + 128, :]
    IDN = np.eye(128, dtype=f32)
    return WIN4, RNT, IDN, IDN.astype(np.float16)


def _build_in_maps(ex, gain, biquads):
    WIN4, RNT, IDN, IDH = _host_consts()
    in_maps = []
    for ci in range(NCORE):
        sl = slice(ci * NB, (ci + 1) * NB)
        exh, C21a, C21b, C21p, GSm, xs0 = _host_prep(
            ex[sl], gain[sl], biquads[sl])
        in_maps.append({
            "exh": exh, "c21a": C21a, "c21b": C21b, "c21p": C21p, "gs": GSm,
            "xs0": xs0,
            "win4": WIN4, "rnt": RNT, "idn": IDN, "idh": IDH,
        })
    return in_maps


def kernel(ex, gain, biquads):
    from concourse.bass_utils import run_bass_kernel_spmd

    ex = np.asarray(ex, np.float32)
    gain = np.asarray(gain, np.float32)
    biquads = np.asarray(biquads, np.float32)

    if "nc" not in _CACHE:
        _CACHE["nc"] = _build_module()
    nc = _CACHE["nc"]

    in_maps = _build_in_maps(ex, gain, biquads)
    res = run_bass_kernel_spmd(nc, in_maps, list(range(NCORE)))
    out = np.concatenate([res.results[ci]["out"] for ci in range(NCORE)], axis=0)
    return out.astype(np.float32)
